# revision 2
# baseline (speedup 1.0000x reference)
"""Trainium2 Bass kernel for nn_Attention_65644280152570.

Dual attention: channel cross-attention (C=2048) produces shared K/V tokens
for 4 spatial multi-head (H=8) cross-attention branches.

Sharding (8 cores): core = 4*h + b with b=batch, h=branch-pair. Each core
computes the full channel branch for its batch (replicated between the two
cores sharing a batch) plus 2 of the 4 spatial branches.

Wire-optimized input distribution (the axon host->device tunnel is ~66MB/s,
so shipped bytes dominate wall time): every input byte is shipped exactly
once where possible. Each core receives ONE packed bf16 blob [3392, 1024]:
  rows [   0,1024): emb_C[b]^T  [2048x512]  (private; x2 across the pair)
  rows [1024,1280): emb_{2h}[b]^T   [512x512] (private, x1)
  rows [1280,1536): emb_{2h+1}[b]^T [512x512] (private, x1)
  rows [1536,1792): branch-weight contribution [512x512] (quad-AllGather, x1)
  rows [1792,3392): 1/8 shard of the shared weight pack (8-way AllGather, x1)
The weight pack [6400, 2048] holds WqC^T, WkC^T, WvC^T, Wk^T, Wv^T; the quad
gather over [[0,1,2,3],[4,5,6,7]] assembles [Wq_{2h}^T, Wq_{2h+1}^T,
Wo_{2h}^T, Wo_{2h+1}^T] (rank r=batch contributes tensor r, so every core
reads the gathered buffer at fixed offsets). Outputs are bf16 (halves d2h).

All matmuls bf16 with f32 PSUM accumulation.

Softmax trick: softmax(inorm(x)) == softmax(x * rsqrt(var(x)+eps)) (the mean
shift cancels row-wise), and logits are ~N(0,1) after scaling so no max
subtraction is needed. Attention maps are kept transposed ([keys, queries])
so the softmax axis sits on partitions and feeds the context matmul
contraction directly; column sums come from ones-augmented matmuls.
"""

import sys
import numpy as np

for p in ("/opt/trn_rl_repo", "/root/.axon_site/_ro/trn_rl_repo"):
    if p not in sys.path:
        sys.path.insert(0, p)

import ml_dtypes

B, N, E, H = 4, 512, 512, 8
C = 4 * E          # 2048
D = E // H         # 64
P = 128
NT = N // P        # 4 n-tiles
CT = C // P        # 16 c/d tiles
ET = E // P        # 4 e-tiles
MT = (4 * N) // P  # 16 token tiles
EPS = 1e-5
M_CH = float(C * C)        # channel inorm element count
M_SP = float(N * 4 * N)    # spatial inorm element count per head

# blob geometry (bf16, width 1024); row offsets of each section
R_EMBC = 0          # 1024 rows: embcT [2048, 512]
R_E0 = 1024         # 256 rows: e0T [512, 512]
R_E1 = 1280         # 256 rows: e1T [512, 512]
R_WBR = 1536        # 256 rows: quad-gather contribution [512, 512]
R_WSH = 1792        # 1600 rows: weight-pack shard [800, 2048]
BLOB_ROWS = 3392
# weight pack row offsets (width 2048)
W_QC = 0            # [2048, 2048]
W_KC = 2048
W_VC = 4096
W_K = 6144          # [512, 512] stored as [128, 2048]
W_V = 6272
WPACK_ROWS = 6400

BF16 = "bfloat16"
_cache = {}


def _build():
    import concourse.bass as bass
    import concourse.mybir as mybir
    import concourse.tile as tile
    from concourse import bacc

    f32 = mybir.dt.float32
    bf16 = mybir.dt.bfloat16
    AX = mybir.AxisListType.X
    ADD = mybir.AluOpType.add
    MULT = mybir.AluOpType.mult
    SUB = mybir.AluOpType.subtract
    BYP = mybir.AluOpType.bypass
    AF = mybir.ActivationFunctionType

    nc = bacc.Bacc("TRN2", target_bir_lowering=False, debug=False, num_devices=8)

    blob_d = nc.dram_tensor("blob", [BLOB_ROWS, 1024], bf16, kind="ExternalInput")
    out_d = nc.dram_tensor("out", [2, N, E], bf16, kind="ExternalOutput")

    def half(sl_rows, b=2):
        # [r, 1024]-rowspace view -> [(r*b), 2048//b] logical rows
        return sl_rows.rearrange("a (b c) -> (a b) c", b=b)

    with tile.TileContext(nc) as tc:
        import contextlib
        ctx = contextlib.ExitStack()
        with ctx:
            const = ctx.enter_context(tc.tile_pool(name="const", bufs=1))
            wpool = ctx.enter_context(tc.tile_pool(name="wpool", bufs=1))
            ps = ctx.enter_context(tc.tile_pool(name="ps", bufs=8, space="PSUM"))
            big = ctx.enter_context(tc.tile_pool(name="big", bufs=1))
            sm = ctx.enter_context(tc.tile_pool(name="sm", bufs=1))
            scr = ctx.enter_context(tc.tile_pool(name="scr", bufs=1))
            dram = ctx.enter_context(tc.tile_pool(name="dram", bufs=2, space="DRAM"))

            # ---------------- collective prologue: distribute weights ------
            wsh_i = dram.tile([800, 2048], bf16, tag="wshi", name="wshi")
            wpg = dram.tile([WPACK_ROWS, 2048], bf16, tag="wpg", name="wpg",
                            addr_space="Shared")
            wbr_i = dram.tile([E, E], bf16, tag="wbri", name="wbri")
            wbrg = dram.tile([4 * E, E], bf16, tag="wbrg", name="wbrg")
            nc.gpsimd.dma_start(wsh_i[:], half(blob_d[R_WSH:BLOB_ROWS, :]))
            nc.gpsimd.dma_start(wbr_i[:], half(blob_d[R_WBR:R_WSH, :]))
            nc.gpsimd.collective_compute(
                "AllGather", BYP, replica_groups=[list(range(8))],
                ins=[wsh_i.opt()], outs=[wpg.opt()])
            nc.gpsimd.collective_compute(
                "AllGather", BYP, replica_groups=[[0, 1, 2, 3], [4, 5, 6, 7]],
                ins=[wbr_i.opt()], outs=[wbrg.opt()])

            ones_col = const.tile([P, 1], bf16, tag="oc", name="oc")
            nc.any.memset(ones_col[:], 1.0)
            ones_col_f = const.tile([P, 1], f32, tag="ocf", name="ocf")
            nc.any.memset(ones_col_f[:], 1.0)
            ones_row_f = const.tile([1, P], f32, tag="orf", name="orf")
            nc.any.memset(ones_row_f[:], 1.0)
            ones_row64 = const.tile([1, D], bf16, tag="or64", name="or64")
            nc.any.memset(ones_row64[:], 1.0)
            eps11 = const.tile([1, 1], f32, tag="eps11", name="eps11")
            nc.any.memset(eps11[:], EPS)

            def psum(p_, n_):
                return ps.tile([p_, n_], f32, tag="ps", name="ps")

            # f32 cross-partition sum: [128,1] f32 -> [1,1] f32 in psum, evict
            def part_sum(src_col, out11):
                pt = psum(1, 1)
                nc.tensor.matmul(pt[:], ones_col_f[:], src_col, start=True, stop=True)
                nc.scalar.copy(out11, pt[:])

            # broadcast [1,1] f32 -> [128,1] f32 (K=1 matmul)
            def bcast_col(src11, out_col):
                pt = psum(P, 1)
                nc.tensor.matmul(pt[:], ones_row_f[:], src11, start=True, stop=True)
                nc.scalar.copy(out_col, pt[:])

            # ---------------- stage A: load embcT, compute QC, KC, VCT ----
            embcT = [big.tile([P, N], bf16, tag="embva", name="embcT", bufs=16, padded_shape=[P, 528]) for _ in range(CT)]
            for kt in range(CT):
                nc.sync.dma_start(embcT[kt][:], half(blob_d[R_EMBC + kt * 64:R_EMBC + (kt + 1) * 64, :]))

            qc = [big.tile([P, C], bf16, tag="qc", name="qc", bufs=4) for _ in range(NT)]
            kc = [big.tile([P, C], bf16, tag="kc", name="kc", bufs=4) for _ in range(NT)]
            for woff, dst in ((W_QC, qc), (W_KC, kc)):
                for ch in range(4):
                    pts = [psum(P, 512) for _ in range(NT)]
                    for kt in range(CT):
                        wt = wpool.tile([P, 512], bf16, tag="wck", name="wck", bufs=3)
                        nc.sync.dma_start(wt[:], wpg[woff + kt * P:woff + (kt + 1) * P, ch * 512:(ch + 1) * 512])
                        for nt in range(NT):
                            nc.tensor.matmul(pts[nt][:], embcT[kt][:, nt * P:(nt + 1) * P],
                                             wt[:], start=(kt == 0), stop=(kt == CT - 1))
                    for nt in range(NT):
                        nc.vector.tensor_copy(dst[nt][:, ch * 512:(ch + 1) * 512], pts[nt][:])

            vct = [big.tile([P, N], bf16, tag="vct", name="vct", bufs=16) for _ in range(CT)]
            for dtg in range(4):
                pts = [psum(P, N) for _ in range(4)]
                for kt in range(CT):
                    wt = wpool.tile([P, 512], bf16, tag="wvk", name="wvk", bufs=3)
                    nc.sync.dma_start(wt[:], wpg[W_VC + kt * P:W_VC + (kt + 1) * P, dtg * 512:(dtg + 1) * 512])
                    for q in range(4):
                        nc.tensor.matmul(pts[q][:], wt[:, q * P:(q + 1) * P], embcT[kt][:],
                                         start=(kt == 0), stop=(kt == CT - 1))
                for q in range(4):
                    nc.vector.tensor_copy(vct[dtg * 4 + q][:], pts[q][:])

            # ---------------- channel attention: A' = attn^T [d, c] -------
            # A' chunks -> DRAM (SBUF can't hold 16MB of A' and E'); global
            # stats accumulate on the fly.
            apd = dram.tile([C, C], bf16, tag="apd", name="apd")
            epd = dram.tile([C, C], bf16, tag="epd", name="epd")
            smsl = sm.tile([P, 64], f32, tag="smsl", name="smsl")
            sqsl = sm.tile([P, 64], f32, tag="sqsl", name="sqsl")
            for dt in range(CT):
                for ch in range(4):
                    pa = psum(P, 512)
                    for nt in range(NT):
                        nc.tensor.matmul(pa[:], kc[nt][:, dt * P:(dt + 1) * P],
                                         qc[nt][:, ch * 512:(ch + 1) * 512],
                                         start=(nt == 0), stop=(nt == NT - 1))
                    idx = dt * 4 + ch
                    sqs = scr.tile([P, 512], bf16, tag="sqs", name="sqs", bufs=2)
                    nc.scalar.activation(sqs[:], pa[:], AF.Square,
                                         accum_out=sqsl[:, idx:idx + 1])
                    apw = scr.tile([P, 512], bf16, tag="apw", name="apw", bufs=3)
                    with nc.allow_low_precision(reason="bf16 evict, f32 accum"):
                        nc.vector.tensor_scalar(apw[:], pa[:], 0.0, 0.0, op0=ADD, op1=ADD,
                                                accum_out=smsl[:, idx:idx + 1])
                    nc.sync.dma_start(apd[dt * P:(dt + 1) * P, ch * 512:(ch + 1) * 512], apw[:])

            # stats -> scale s = 1/sqrt(var+eps), broadcast to [128,1]
            smv = sm.tile([P, 1], f32, tag="smv", name="smv")
            sqv = sm.tile([P, 1], f32, tag="sqv", name="sqv")
            nc.vector.tensor_reduce(smv[:], smsl[:], AX, ADD)
            nc.vector.tensor_reduce(sqv[:], sqsl[:], AX, ADD)
            stot = sm.tile([1, 1], f32, tag="stot", name="stot")
            qtot = sm.tile([1, 1], f32, tag="qtot", name="qtot")
            part_sum(smv[:], stot[:])
            part_sum(sqv[:], qtot[:])
            m2 = sm.tile([1, 1], f32, tag="m2", name="m2")
            t2 = sm.tile([1, 1], f32, tag="t2", name="t2")
            nc.scalar.activation(m2[:], stot[:], AF.Square, scale=1.0 / M_CH)
            nc.scalar.activation(t2[:], qtot[:], AF.Copy, scale=1.0 / M_CH)
            var1 = sm.tile([1, 1], f32, tag="var1", name="var1")
            nc.vector.tensor_tensor(var1[:], t2[:], m2[:], op=SUB)
            sd1 = sm.tile([1, 1], f32, tag="sd1", name="sd1")
            nc.scalar.activation(sd1[:], var1[:], AF.Sqrt, bias=eps11[:])
            s11 = sm.tile([1, 1], f32, tag="s11", name="s11")
            nc.vector.reciprocal(s11[:], sd1[:])
            sbc = sm.tile([P, 1], f32, tag="sbc", name="sbc")
            bcast_col(s11[:], sbc[:])

            # pass A: stream A' from DRAM, exp, accumulate column sums over
            # d (partitions, via ones-lhsT matmul); write E' back to DRAM
            pcs = [psum(1, 512) for _ in range(4)]
            for dt in range(CT):
                apr = scr.tile([P, C], bf16, tag="apr", name="apr", bufs=3)
                nc.sync.dma_start(apr[:], apd[dt * P:(dt + 1) * P, :])
                nc.scalar.activation(apr[:], apr[:], AF.Exp, scale=sbc[:])
                for ch in range(4):
                    nc.tensor.matmul(pcs[ch][:], ones_col[:],
                                     apr[:, ch * 512:(ch + 1) * 512],
                                     start=(dt == 0), stop=(dt == CT - 1))
                nc.sync.dma_start(epd[dt * P:(dt + 1) * P, :], apr[:])
            rr = sm.tile([1, C], f32, tag="rr", name="rr")
            for ch in range(4):
                nc.vector.reciprocal(rr[:, ch * 512:(ch + 1) * 512], pcs[ch][:])
            # transpose [1, C] -> [128, 16] via DRAM bounce
            rb_d = dram.tile([1, C], f32, tag="rb", name="rb")
            nc.sync.dma_start(rb_d[:], rr[:])
            rT = sm.tile([P, CT], f32, tag="rT", name="rT")
            nc.sync.dma_start(rT[:], rb_d[:].rearrange("a (t p) -> (a p) t", p=P))

            # pass B: ctx[c,n] = (E'^T @ VCT) * recip_colsum[c], two groups of
            # 8 PSUM accumulators; E' streamed per d-tile
            ctx_sb = [big.tile([P, N], bf16, tag="ctx", name="ctx", bufs=16) for _ in range(CT)]
            for g in range(2):
                pcxs = [psum(P, N) for _ in range(8)]
                for dt in range(CT):
                    epr = scr.tile([P, C], bf16, tag="apr", name="epr", bufs=3)
                    nc.sync.dma_start(epr[:], epd[dt * P:(dt + 1) * P, :])
                    for k in range(8):
                        ct = g * 8 + k
                        nc.tensor.matmul(pcxs[k][:], epr[:, ct * P:(ct + 1) * P], vct[dt][:],
                                         start=(dt == 0), stop=(dt == CT - 1))
                for k in range(8):
                    ct = g * 8 + k
                    nc.vector.tensor_scalar_mul(ctx_sb[ct][:], pcxs[k][:], rT[:, ct:ct + 1])

            # ---------------- shared K/V over the 4N gathered tokens ------
            wk_sb = [sm.tile([P, E], bf16, tag="wk", name="wk", bufs=4) for _ in range(ET)]
            wv_sb = [sm.tile([P, E], bf16, tag="wv", name="wv", bufs=4) for _ in range(ET)]
            for et in range(ET):
                nc.sync.dma_start(wk_sb[et][:], half(wpg[W_K + et * 32:W_K + (et + 1) * 32, :], b=4))
                nc.sync.dma_start(wv_sb[et][:], half(wpg[W_V + et * 32:W_V + (et + 1) * 32, :], b=4))

            kt_sb = [big.tile([P, 4 * N], bf16, tag="kt", name="kt", bufs=4) for _ in range(ET)]
            for pt in range(ET):
                for j in range(4):
                    pk = psum(P, 512)
                    for et in range(ET):
                        nc.tensor.matmul(pk[:], wk_sb[et][:, pt * P:(pt + 1) * P],
                                         ctx_sb[4 * j + et][:],
                                         start=(et == 0), stop=(et == ET - 1))
                    nc.vector.tensor_copy(kt_sb[pt][:, j * 512:(j + 1) * 512], pk[:])

            vaug = [big.tile([P, H * (D + 1)], bf16, tag="embva", name="vaug", bufs=16, padded_shape=[P, 528]) for _ in range(MT)]
            for mt in range(MT):
                j, q = mt // 4, mt % 4
                pv = psum(P, 512)
                for et in range(ET):
                    nc.tensor.matmul(pv[:], ctx_sb[4 * j + et][:, q * P:(q + 1) * P],
                                     wv_sb[et][:], start=(et == 0), stop=(et == ET - 1))
                va = vaug[mt][:].rearrange("p (h x) -> p h x", x=D + 1)
                nc.vector.tensor_copy(va[:, :, 0:D], pv[:].rearrange("p (h x) -> p h x", x=D))
                nc.any.memset(va[:, :, D:D + 1], 1.0)

            # ---------------- two spatial branches -------------------------
            for br in range(2):
                ebT = [sm.tile([P, N], bf16, tag="ebT", name="ebT", bufs=4) for _ in range(ET)]
                wq_sb = [sm.tile([P, E], bf16, tag="wq", name="wq", bufs=4) for _ in range(ET)]
                wo_sb = [sm.tile([P, E], bf16, tag="wo", name="wo", bufs=4) for _ in range(ET)]
                r_e = R_E0 if br == 0 else R_E1
                for et in range(ET):
                    nc.sync.dma_start(ebT[et][:], half(blob_d[r_e + et * 64:r_e + (et + 1) * 64, :]))
                    nc.sync.dma_start(wq_sb[et][:], wbrg[br * 512 + et * P:br * 512 + (et + 1) * P, :])
                    nc.sync.dma_start(wo_sb[et][:], wbrg[1024 + br * 512 + et * P:1024 + br * 512 + (et + 1) * P, :])

                qt_sb = [sm.tile([P, N], bf16, tag="qt", name="qt", bufs=4) for _ in range(ET)]
                for pt in range(ET):
                    pq = psum(P, N)
                    for et in range(ET):
                        nc.tensor.matmul(pq[:], wq_sb[et][:, pt * P:(pt + 1) * P],
                                         ebT[et][:], start=(et == 0), stop=(et == ET - 1))
                    nc.vector.tensor_copy(qt_sb[pt][:], pq[:])

                ctxT = [sm.tile([P, N], bf16, tag="ctxT", name="ctxT", bufs=8) for _ in range(ET)]
                for h in range(H):
                    pt, off = h // 2, (h % 2) * D
                    lh = big.tile([P, MT * N], bf16, tag="lh", name="lh", bufs=2)
                    hsm = sm.tile([P, MT], f32, tag="hsm", name="hsm", bufs=2)
                    hsq = sm.tile([P, MT], f32, tag="hsq", name="hsq", bufs=2)
                    for mt in range(MT):
                        pl = psum(P, N)
                        nc.tensor.matmul(pl[:], kt_sb[pt][off:off + D, mt * P:(mt + 1) * P],
                                         qt_sb[pt][off:off + D, :], start=True, stop=True)
                        sqs = scr.tile([P, 512], bf16, tag="sqs", name="sqs", bufs=2)
                        nc.scalar.activation(sqs[:], pl[:], AF.Square,
                                             accum_out=hsq[:, mt:mt + 1])
                        with nc.allow_low_precision(reason="bf16 evict, f32 accum"):
                            nc.vector.tensor_scalar(lh[:, mt * N:(mt + 1) * N], pl[:],
                                                    0.0, 0.0, op0=ADD, op1=ADD,
                                                    accum_out=hsm[:, mt:mt + 1])
                    hsmv = sm.tile([P, 1], f32, tag="hsmv", name="hsmv", bufs=2)
                    hsqv = sm.tile([P, 1], f32, tag="hsqv", name="hsqv", bufs=2)
                    nc.vector.tensor_reduce(hsmv[:], hsm[:], AX, ADD)
                    nc.vector.tensor_reduce(hsqv[:], hsq[:], AX, ADD)
                    hst = sm.tile([1, 1], f32, tag="hst", name="hst", bufs=2)
                    hqt = sm.tile([1, 1], f32, tag="hqt", name="hqt", bufs=2)
                    part_sum(hsmv[:], hst[:])
                    part_sum(hsqv[:], hqt[:])
                    hm2 = sm.tile([1, 1], f32, tag="hm2", name="hm2", bufs=2)
                    ht2 = sm.tile([1, 1], f32, tag="ht2", name="ht2", bufs=2)
                    nc.scalar.activation(hm2[:], hst[:], AF.Square, scale=1.0 / M_SP)
                    nc.scalar.activation(ht2[:], hqt[:], AF.Copy, scale=1.0 / M_SP)
                    hvar = sm.tile([1, 1], f32, tag="hvar", name="hvar", bufs=2)
                    nc.vector.tensor_tensor(hvar[:], ht2[:], hm2[:], op=SUB)
                    hsd1 = sm.tile([1, 1], f32, tag="hsd1", name="hsd1", bufs=2)
                    nc.scalar.activation(hsd1[:], hvar[:], AF.Sqrt, bias=eps11[:])
                    hs11 = sm.tile([1, 1], f32, tag="hs11", name="hs11", bufs=2)
                    nc.vector.reciprocal(hs11[:], hsd1[:])
                    hsbc = sm.tile([P, 1], f32, tag="hsbc", name="hsbc", bufs=2)
                    bcast_col(hs11[:], hsbc[:])

                    nc.scalar.activation(lh[:], lh[:], AF.Exp, scale=hsbc[:])
                    es = lh

                    pcx2 = ps.tile([D + 1, N], f32, tag="ps", name="ps")
                    for mt in range(MT):
                        nc.tensor.matmul(pcx2[:], vaug[mt][:, h * (D + 1):(h + 1) * (D + 1)],
                                         es[:, mt * N:(mt + 1) * N],
                                         start=(mt == 0), stop=(mt == MT - 1))
                    rcs = sm.tile([1, N], bf16, tag="rcs", name="rcs", bufs=2)
                    with nc.allow_low_precision(reason="bf16 reciprocal for bcast matmul"):
                        nc.vector.reciprocal(rcs[:], pcx2[D:D + 1, :])
                    prb = psum(D, N)
                    nc.tensor.matmul(prb[:], ones_row64[:], rcs[:], start=True, stop=True)
                    rcb = sm.tile([D, N], f32, tag="rcb", name="rcb", bufs=2)
                    nc.scalar.copy(rcb[:], prb[:])
                    nc.vector.tensor_tensor(ctxT[pt][off:off + D, :], pcx2[0:D, :],
                                            rcb[:], op=MULT)

                for nt2 in range(NT):
                    po = psum(P, E)
                    for pt in range(ET):
                        nc.tensor.matmul(po[:], ctxT[pt][:, nt2 * P:(nt2 + 1) * P],
                                         wo_sb[pt][:], start=(pt == 0), stop=(pt == ET - 1))
                    osb = scr.tile([P, E], bf16, tag="osb", name="osb", bufs=2)
                    nc.vector.tensor_copy(osb[:], po[:])
                    nc.sync.dma_start(out_d[br, nt2 * P:(nt2 + 1) * P, :], osb[:])

    nc.compile()
    return nc


def _get_nc():
    if "nc" not in _cache:
        _cache["nc"] = _build()
    return _cache["nc"]


def kernel(emb1, emb2, emb3, emb4, emb_C,
           Wq1, Wq2, Wq3, Wq4, Wk, Wv, WqC, WkC, WvC,
           Wo1, Wo2, Wo3, Wo4):
    from concourse.bass_utils import run_bass_kernel_spmd

    bf = ml_dtypes.bfloat16
    embs = [np.asarray(e, np.float32) for e in (emb1, emb2, emb3, emb4)]
    Wqs = [np.asarray(w, np.float32) for w in (Wq1, Wq2, Wq3, Wq4)]
    Wos = [np.asarray(w, np.float32) for w in (Wo1, Wo2, Wo3, Wo4)]
    emb_C = np.asarray(emb_C, np.float32)
    WqC, WkC, WvC = (np.asarray(w, np.float32) for w in (WqC, WkC, WvC))
    Wk, Wv = np.asarray(Wk, np.float32), np.asarray(Wv, np.float32)

    blob = np.empty((8, BLOB_ROWS, 1024), bf)

    # shared weight pack -> 8-way shards
    wpack = np.empty((WPACK_ROWS, 2048), bf)
    wpack[W_QC:W_QC + C] = WqC.T
    wpack[W_KC:W_KC + C] = WkC.T
    wpack[W_VC:W_VC + C] = WvC.T
    wpack[W_K:W_K + 128].reshape(E, E)[:] = Wk.T
    wpack[W_V:W_V + 128].reshape(E, E)[:] = Wv.T
    blob[:, R_WSH:BLOB_ROWS, :] = wpack.reshape(8, 1600, 1024)

    for b in range(B):
        ecT = emb_C[b].T.astype(bf)  # [2048, 512]
        for h in range(2):
            c = 4 * h + b
            blob[c, R_EMBC:R_E0, :].reshape(C, N)[:] = ecT
            blob[c, R_E0:R_E1, :].reshape(E, N)[:] = embs[2 * h][b].T
            blob[c, R_E1:R_WBR, :].reshape(E, N)[:] = embs[2 * h + 1][b].T
            # quad-gather contribution: rank b of [[0..3],[4..7]] ships
            # [Wq_2h, Wq_2h+1, Wo_2h, Wo_2h+1][b]
            wsrc = (Wqs[2 * h], Wqs[2 * h + 1], Wos[2 * h], Wos[2 * h + 1])[b]
            blob[c, R_WBR:R_WSH, :].reshape(E, E)[:] = wsrc.T

    in_maps = [{"blob": blob[c]} for c in range(8)]

    import os
    nc = _get_nc()
    trace = bool(os.environ.get("BASSK_TRACE"))
    try:
        res = run_bass_kernel_spmd(nc, in_maps, core_ids=list(range(8)), trace=trace)
    except ModuleNotFoundError:
        # NTFF profile hook unavailable in this container; run untraced
        res = run_bass_kernel_spmd(nc, in_maps, core_ids=list(range(8)))
    _cache["last_result"] = res

    outs = []
    for br in range(4):
        h, j = br // 2, br % 2
        outs.append(np.stack([res.results[4 * h + b]["out"][j].astype(np.float32)
                              for b in range(B)]))
    return tuple(outs)


if __name__ == "__main__":
    sys.path.insert(0, "/root/problem")
    import reference
    inputs = reference.setup_inputs()
    inputs = {k: np.asarray(v) for k, v in inputs.items()}
    exp = reference.reference(**inputs)
    act = kernel(**inputs)
    for i, (a, e) in enumerate(zip(act, exp)):
        e = np.asarray(e)
        err = np.linalg.norm(a - e) / max(np.linalg.norm(e), 1e-30)
        print(f"out{i + 1}: rel_err={err:.3e}")


# revision 6
# speedup vs baseline: 1.0288x; 1.0288x over previous
"""Trainium2 Bass kernel for nn_Attention_65644280152570.

Dual attention: channel cross-attention (C=2048) produces shared K/V tokens
for 4 spatial multi-head (H=8) cross-attention branches.

Sharding (8 cores): core = 4*h + b with b=batch, h=branch-pair. Each core
computes the full channel branch for its batch (replicated between the two
cores sharing a batch) plus 2 of the 4 spatial branches.

Wire-optimized input distribution (the axon host->device tunnel is ~66MB/s,
so shipped bytes dominate wall time): every input byte is shipped exactly
once where possible. Each core receives ONE packed bf16 blob [3392, 1024]:
  rows [   0,1024): emb_C[b]^T  [2048x512]  (private; x2 across the pair)
  rows [1024,1280): emb_{2h}[b]^T   [512x512] (private, x1)
  rows [1280,1536): emb_{2h+1}[b]^T [512x512] (private, x1)
  rows [1536,1792): branch-weight contribution [512x512] (quad-AllGather, x1)
  rows [1792,3392): 1/8 shard of the shared weight pack (8-way AllGather, x1)
The weight pack [6400, 2048] holds WqC^T, WkC^T, WvC^T, Wk^T, Wv^T; the quad
gather over [[0,1,2,3],[4,5,6,7]] assembles [Wq_{2h}^T, Wq_{2h+1}^T,
Wo_{2h}^T, Wo_{2h+1}^T] (rank r=batch contributes tensor r, so every core
reads the gathered buffer at fixed offsets). Outputs are bf16 (halves d2h).

All matmuls bf16 with f32 PSUM accumulation.

Softmax trick: softmax(inorm(x)) == softmax(x * rsqrt(var(x)+eps)) (the mean
shift cancels row-wise), and logits are ~N(0,1) after scaling so no max
subtraction is needed. Attention maps are kept transposed ([keys, queries])
so the softmax axis sits on partitions and feeds the context matmul
contraction directly; column sums come from ones-augmented matmuls.
"""

import sys
import numpy as np

for p in ("/opt/trn_rl_repo", "/root/.axon_site/_ro/trn_rl_repo"):
    if p not in sys.path:
        sys.path.insert(0, p)

import ml_dtypes

B, N, E, H = 4, 512, 512, 8
C = 4 * E          # 2048
D = E // H         # 64
P = 128
NT = N // P        # 4 n-tiles
CT = C // P        # 16 c/d tiles
ET = E // P        # 4 e-tiles
MT = (4 * N) // P  # 16 token tiles
EPS = 1e-5
M_CH = float(C * C)        # channel inorm element count
M_SP = float(N * 4 * N)    # spatial inorm element count per head

# blob geometry (bf16, width 1024); row offsets of each section
R_EMBC = 0          # 1024 rows: embcT [2048, 512]
R_E0 = 1024         # 256 rows: e0T [512, 512]
R_E1 = 1280         # 256 rows: e1T [512, 512]
R_WBR = 1536        # 256 rows: quad-gather contribution [512, 512]
R_WSH = 1792        # 1600 rows: weight-pack shard [800, 2048]
BLOB_ROWS = 3392
# weight pack row offsets (width 2048)
W_QC = 0            # [2048, 2048]
W_KC = 2048
W_VC = 4096
W_K = 6144          # [512, 512] stored as [128, 2048]
W_V = 6272
WPACK_ROWS = 6400

BF16 = "bfloat16"
_cache = {}


def _build():
    import concourse.bass as bass
    import concourse.mybir as mybir
    import concourse.tile as tile
    from concourse import bacc

    f32 = mybir.dt.float32
    bf16 = mybir.dt.bfloat16
    AX = mybir.AxisListType.X
    ADD = mybir.AluOpType.add
    MULT = mybir.AluOpType.mult
    SUB = mybir.AluOpType.subtract
    BYP = mybir.AluOpType.bypass
    AF = mybir.ActivationFunctionType

    nc = bacc.Bacc("TRN2", target_bir_lowering=False, debug=False, num_devices=8)

    blob_d = nc.dram_tensor("blob", [BLOB_ROWS, 1024], bf16, kind="ExternalInput")
    out_d = nc.dram_tensor("out", [2, N, E], bf16, kind="ExternalOutput")

    def half(sl_rows, b=2):
        # [r, 1024]-rowspace view -> [(r*b), 2048//b] logical rows
        return sl_rows.rearrange("a (b c) -> (a b) c", b=b)

    with tile.TileContext(nc) as tc:
        import contextlib
        ctx = contextlib.ExitStack()
        with ctx:
            const = ctx.enter_context(tc.tile_pool(name="const", bufs=1))
            wpool = ctx.enter_context(tc.tile_pool(name="wpool", bufs=1))
            ps = ctx.enter_context(tc.tile_pool(name="ps", bufs=8, space="PSUM"))
            big = ctx.enter_context(tc.tile_pool(name="big", bufs=1))
            sm = ctx.enter_context(tc.tile_pool(name="sm", bufs=1))
            scr = ctx.enter_context(tc.tile_pool(name="scr", bufs=1))
            dram = ctx.enter_context(tc.tile_pool(name="dram", bufs=2, space="DRAM"))

            # ---------------- collective prologue: distribute weights ------
            wsh_i = dram.tile([800, 2048], bf16, tag="wshi", name="wshi")
            wpg = dram.tile([WPACK_ROWS, 2048], bf16, tag="wpg", name="wpg",
                            addr_space="Shared")
            wbr_i = dram.tile([E, E], bf16, tag="wbri", name="wbri")
            wbrg = dram.tile([4 * E, E], bf16, tag="wbrg", name="wbrg")
            nc.gpsimd.dma_start(wsh_i[:], half(blob_d[R_WSH:BLOB_ROWS, :]))
            nc.gpsimd.dma_start(wbr_i[:], half(blob_d[R_WBR:R_WSH, :]))
            nc.gpsimd.collective_compute(
                "AllGather", BYP, replica_groups=[list(range(8))],
                ins=[wsh_i.opt()], outs=[wpg.opt()])
            nc.gpsimd.collective_compute(
                "AllGather", BYP, replica_groups=[[0, 1, 2, 3], [4, 5, 6, 7]],
                ins=[wbr_i.opt()], outs=[wbrg.opt()])

            ones_col = const.tile([P, 1], bf16, tag="oc", name="oc")
            nc.any.memset(ones_col[:], 1.0)
            ones_col_f = const.tile([P, 1], f32, tag="ocf", name="ocf")
            nc.any.memset(ones_col_f[:], 1.0)
            ones_row_f = const.tile([1, P], f32, tag="orf", name="orf")
            nc.any.memset(ones_row_f[:], 1.0)
            ones_row64 = const.tile([1, D], bf16, tag="or64", name="or64")
            nc.any.memset(ones_row64[:], 1.0)
            eps11 = const.tile([1, 1], f32, tag="eps11", name="eps11")
            nc.any.memset(eps11[:], EPS)

            def psum(p_, n_):
                return ps.tile([p_, n_], f32, tag="ps", name="ps")

            # f32 cross-partition sum: [128,1] f32 -> [1,1] f32 in psum, evict
            def part_sum(src_col, out11):
                pt = psum(1, 1)
                nc.tensor.matmul(pt[:], ones_col_f[:], src_col, start=True, stop=True)
                nc.scalar.copy(out11, pt[:])

            # broadcast [1,1] f32 -> [128,1] f32 (K=1 matmul)
            def bcast_col(src11, out_col):
                pt = psum(P, 1)
                nc.tensor.matmul(pt[:], ones_row_f[:], src11, start=True, stop=True)
                nc.scalar.copy(out_col, pt[:])

            # ---------------- stage A: load embcT, compute QC, KC, VCT ----
            embcT = [big.tile([P, N], bf16, tag="embva", name="embcT", bufs=16, padded_shape=[P, 528]) for _ in range(CT)]
            for kt in range(CT):
                nc.sync.dma_start(embcT[kt][:], half(blob_d[R_EMBC + kt * 64:R_EMBC + (kt + 1) * 64, :]))

            qc = [big.tile([P, C], bf16, tag="qc", name="qc", bufs=4) for _ in range(NT)]
            kc = [big.tile([P, C], bf16, tag="kc", name="kc", bufs=4) for _ in range(NT)]
            for woff, dst in ((W_QC, qc), (W_KC, kc)):
                for ch in range(4):
                    pts = [psum(P, 512) for _ in range(NT)]
                    for kt in range(CT):
                        wt = wpool.tile([P, 512], bf16, tag="wck", name="wck", bufs=3)
                        nc.sync.dma_start(wt[:], wpg[woff + kt * P:woff + (kt + 1) * P, ch * 512:(ch + 1) * 512])
                        for nt in range(NT):
                            nc.tensor.matmul(pts[nt][:], embcT[kt][:, nt * P:(nt + 1) * P],
                                             wt[:], start=(kt == 0), stop=(kt == CT - 1))
                    for nt in range(NT):
                        nc.vector.tensor_copy(dst[nt][:, ch * 512:(ch + 1) * 512], pts[nt][:])

            vct = [big.tile([P, N], bf16, tag="vct", name="vct", bufs=16) for _ in range(CT)]
            for dtg in range(4):
                pts = [psum(P, N) for _ in range(4)]
                for kt in range(CT):
                    wt = wpool.tile([P, 512], bf16, tag="wvk", name="wvk", bufs=3)
                    nc.sync.dma_start(wt[:], wpg[W_VC + kt * P:W_VC + (kt + 1) * P, dtg * 512:(dtg + 1) * 512])
                    for q in range(4):
                        nc.tensor.matmul(pts[q][:], wt[:, q * P:(q + 1) * P], embcT[kt][:],
                                         start=(kt == 0), stop=(kt == CT - 1))
                for q in range(4):
                    nc.vector.tensor_copy(vct[dtg * 4 + q][:], pts[q][:])

            # ---------------- channel attention: A' = attn^T [d, c] -------
            # A' chunks -> DRAM (SBUF can't hold 16MB of A' and E'); global
            # stats accumulate on the fly.
            apd = dram.tile([C, C], bf16, tag="apd", name="apd")
            epd = dram.tile([C, C], bf16, tag="epd", name="epd")
            smsl = sm.tile([P, 64], f32, tag="smsl", name="smsl")
            sqsl = sm.tile([P, 64], f32, tag="sqsl", name="sqsl")
            for dt in range(CT):
                for ch in range(4):
                    pa = psum(P, 512)
                    for nt in range(NT):
                        nc.tensor.matmul(pa[:], kc[nt][:, dt * P:(dt + 1) * P],
                                         qc[nt][:, ch * 512:(ch + 1) * 512],
                                         start=(nt == 0), stop=(nt == NT - 1))
                    idx = dt * 4 + ch
                    sqs = scr.tile([P, 512], bf16, tag="sqs", name="sqs", bufs=2)
                    nc.scalar.activation(sqs[:], pa[:], AF.Square,
                                         accum_out=sqsl[:, idx:idx + 1])
                    apw = scr.tile([P, 512], bf16, tag="apw", name="apw", bufs=3)
                    with nc.allow_low_precision(reason="bf16 evict, f32 accum"):
                        nc.vector.tensor_scalar(apw[:], pa[:], 0.0, 0.0, op0=ADD, op1=ADD,
                                                accum_out=smsl[:, idx:idx + 1])
                    nc.sync.dma_start(apd[dt * P:(dt + 1) * P, ch * 512:(ch + 1) * 512], apw[:])

            # stats -> scale s = 1/sqrt(var+eps), broadcast to [128,1]
            smv = sm.tile([P, 1], f32, tag="smv", name="smv")
            sqv = sm.tile([P, 1], f32, tag="sqv", name="sqv")
            nc.vector.tensor_reduce(smv[:], smsl[:], AX, ADD)
            nc.vector.tensor_reduce(sqv[:], sqsl[:], AX, ADD)
            stot = sm.tile([1, 1], f32, tag="stot", name="stot")
            qtot = sm.tile([1, 1], f32, tag="qtot", name="qtot")
            part_sum(smv[:], stot[:])
            part_sum(sqv[:], qtot[:])
            m2 = sm.tile([1, 1], f32, tag="m2", name="m2")
            t2 = sm.tile([1, 1], f32, tag="t2", name="t2")
            nc.scalar.activation(m2[:], stot[:], AF.Square, scale=1.0 / M_CH)
            nc.scalar.activation(t2[:], qtot[:], AF.Copy, scale=1.0 / M_CH)
            var1 = sm.tile([1, 1], f32, tag="var1", name="var1")
            nc.vector.tensor_tensor(var1[:], t2[:], m2[:], op=SUB)
            sd1 = sm.tile([1, 1], f32, tag="sd1", name="sd1")
            nc.scalar.activation(sd1[:], var1[:], AF.Sqrt, bias=eps11[:])
            s11 = sm.tile([1, 1], f32, tag="s11", name="s11")
            nc.vector.reciprocal(s11[:], sd1[:])
            sbc = sm.tile([P, 1], f32, tag="sbc", name="sbc")
            bcast_col(s11[:], sbc[:])

            # pass A: stream A' from DRAM, exp, accumulate column sums over
            # d (partitions, via ones-lhsT matmul); write E' back to DRAM
            pcs = [psum(1, 512) for _ in range(4)]
            for dt in range(CT):
                apr = scr.tile([P, C], bf16, tag="apr", name="apr", bufs=3)
                nc.sync.dma_start(apr[:], apd[dt * P:(dt + 1) * P, :])
                nc.scalar.activation(apr[:], apr[:], AF.Exp, scale=sbc[:])
                for ch in range(4):
                    nc.tensor.matmul(pcs[ch][:], ones_col[:],
                                     apr[:, ch * 512:(ch + 1) * 512],
                                     start=(dt == 0), stop=(dt == CT - 1))
                nc.sync.dma_start(epd[dt * P:(dt + 1) * P, :], apr[:])
            rr = sm.tile([1, C], f32, tag="rr", name="rr")
            for ch in range(4):
                nc.vector.reciprocal(rr[:, ch * 512:(ch + 1) * 512], pcs[ch][:])
            # transpose [1, C] -> [128, 16] via DRAM bounce
            rb_d = dram.tile([1, C], f32, tag="rb", name="rb")
            nc.sync.dma_start(rb_d[:], rr[:])
            rT = sm.tile([P, CT], f32, tag="rT", name="rT")
            nc.sync.dma_start(rT[:], rb_d[:].rearrange("a (t p) -> (a p) t", p=P))

            # pass B: ctx[c,n] = (E'^T @ VCT) * recip_colsum[c], two groups of
            # 8 PSUM accumulators; E' streamed per d-tile
            ctx_sb = [big.tile([P, N], bf16, tag="ctx", name="ctx", bufs=16) for _ in range(CT)]
            for g in range(2):
                pcxs = [psum(P, N) for _ in range(8)]
                for dt in range(CT):
                    epr = scr.tile([P, C], bf16, tag="apr", name="epr", bufs=3)
                    nc.sync.dma_start(epr[:], epd[dt * P:(dt + 1) * P, :])
                    for k in range(8):
                        ct = g * 8 + k
                        nc.tensor.matmul(pcxs[k][:], epr[:, ct * P:(ct + 1) * P], vct[dt][:],
                                         start=(dt == 0), stop=(dt == CT - 1))
                for k in range(8):
                    ct = g * 8 + k
                    nc.vector.tensor_scalar_mul(ctx_sb[ct][:], pcxs[k][:], rT[:, ct:ct + 1])

            # ---------------- shared K/V over the 4N gathered tokens ------
            wk_sb = [sm.tile([P, E], bf16, tag="wk", name="wk", bufs=4) for _ in range(ET)]
            wv_sb = [sm.tile([P, E], bf16, tag="wv", name="wv", bufs=4) for _ in range(ET)]
            for et in range(ET):
                nc.sync.dma_start(wk_sb[et][:], half(wpg[W_K + et * 32:W_K + (et + 1) * 32, :], b=4))
                nc.sync.dma_start(wv_sb[et][:], half(wpg[W_V + et * 32:W_V + (et + 1) * 32, :], b=4))

            kt_sb = [big.tile([P, 4 * N], bf16, tag="kt", name="kt", bufs=4) for _ in range(ET)]
            for pt in range(ET):
                for j in range(4):
                    pk = psum(P, 512)
                    for et in range(ET):
                        nc.tensor.matmul(pk[:], wk_sb[et][:, pt * P:(pt + 1) * P],
                                         ctx_sb[4 * j + et][:],
                                         start=(et == 0), stop=(et == ET - 1))
                    nc.vector.tensor_copy(kt_sb[pt][:, j * 512:(j + 1) * 512], pk[:])

            vaug = [big.tile([P, H * (D + 1)], bf16, tag="embva", name="vaug", bufs=16, padded_shape=[P, 528]) for _ in range(MT)]
            for mt in range(MT):
                j, q = mt // 4, mt % 4
                pv = psum(P, 512)
                for et in range(ET):
                    nc.tensor.matmul(pv[:], ctx_sb[4 * j + et][:, q * P:(q + 1) * P],
                                     wv_sb[et][:], start=(et == 0), stop=(et == ET - 1))
                va = vaug[mt][:].rearrange("p (h x) -> p h x", x=D + 1)
                nc.vector.tensor_copy(va[:, :, 0:D], pv[:].rearrange("p (h x) -> p h x", x=D))
                nc.any.memset(va[:, :, D:D + 1], 1.0)

            # ---------------- two spatial branches -------------------------
            for br in range(2):
                ebT = [sm.tile([P, N], bf16, tag="ebT", name="ebT", bufs=4) for _ in range(ET)]
                wq_sb = [sm.tile([P, E], bf16, tag="wq", name="wq", bufs=4) for _ in range(ET)]
                wo_sb = [sm.tile([P, E], bf16, tag="wo", name="wo", bufs=4) for _ in range(ET)]
                r_e = R_E0 if br == 0 else R_E1
                for et in range(ET):
                    nc.sync.dma_start(ebT[et][:], half(blob_d[r_e + et * 64:r_e + (et + 1) * 64, :]))
                    nc.sync.dma_start(wq_sb[et][:], wbrg[br * 512 + et * P:br * 512 + (et + 1) * P, :])
                    nc.sync.dma_start(wo_sb[et][:], wbrg[1024 + br * 512 + et * P:1024 + br * 512 + (et + 1) * P, :])

                qt_sb = [sm.tile([P, N], bf16, tag="qt", name="qt", bufs=4) for _ in range(ET)]
                for pt in range(ET):
                    pq = psum(P, N)
                    for et in range(ET):
                        nc.tensor.matmul(pq[:], wq_sb[et][:, pt * P:(pt + 1) * P],
                                         ebT[et][:], start=(et == 0), stop=(et == ET - 1))
                    nc.vector.tensor_copy(qt_sb[pt][:], pq[:])

                ctxT = [sm.tile([P, N], bf16, tag="ctxT", name="ctxT", bufs=8) for _ in range(ET)]
                for h in range(H):
                    pt, off = h // 2, (h % 2) * D
                    lh = big.tile([P, MT * N], bf16, tag="lh", name="lh", bufs=2)
                    hsm = sm.tile([P, MT], f32, tag="hsm", name="hsm", bufs=2)
                    hsq = sm.tile([P, MT], f32, tag="hsq", name="hsq", bufs=2)
                    for mt in range(MT):
                        pl = psum(P, N)
                        nc.tensor.matmul(pl[:], kt_sb[pt][off:off + D, mt * P:(mt + 1) * P],
                                         qt_sb[pt][off:off + D, :], start=True, stop=True)
                        sqs = scr.tile([P, 512], bf16, tag="sqs", name="sqs", bufs=2)
                        nc.scalar.activation(sqs[:], pl[:], AF.Square,
                                             accum_out=hsq[:, mt:mt + 1])
                        with nc.allow_low_precision(reason="bf16 evict, f32 accum"):
                            nc.vector.tensor_scalar(lh[:, mt * N:(mt + 1) * N], pl[:],
                                                    0.0, 0.0, op0=ADD, op1=ADD,
                                                    accum_out=hsm[:, mt:mt + 1])
                    hsmv = sm.tile([P, 1], f32, tag="hsmv", name="hsmv", bufs=2)
                    hsqv = sm.tile([P, 1], f32, tag="hsqv", name="hsqv", bufs=2)
                    nc.vector.tensor_reduce(hsmv[:], hsm[:], AX, ADD)
                    nc.vector.tensor_reduce(hsqv[:], hsq[:], AX, ADD)
                    hst = sm.tile([1, 1], f32, tag="hst", name="hst", bufs=2)
                    hqt = sm.tile([1, 1], f32, tag="hqt", name="hqt", bufs=2)
                    part_sum(hsmv[:], hst[:])
                    part_sum(hsqv[:], hqt[:])
                    hm2 = sm.tile([1, 1], f32, tag="hm2", name="hm2", bufs=2)
                    ht2 = sm.tile([1, 1], f32, tag="ht2", name="ht2", bufs=2)
                    nc.scalar.activation(hm2[:], hst[:], AF.Square, scale=1.0 / M_SP)
                    nc.scalar.activation(ht2[:], hqt[:], AF.Copy, scale=1.0 / M_SP)
                    hvar = sm.tile([1, 1], f32, tag="hvar", name="hvar", bufs=2)
                    nc.vector.tensor_tensor(hvar[:], ht2[:], hm2[:], op=SUB)
                    hsd1 = sm.tile([1, 1], f32, tag="hsd1", name="hsd1", bufs=2)
                    nc.scalar.activation(hsd1[:], hvar[:], AF.Sqrt, bias=eps11[:])
                    hs11 = sm.tile([1, 1], f32, tag="hs11", name="hs11", bufs=2)
                    nc.vector.reciprocal(hs11[:], hsd1[:])
                    hsbc = sm.tile([P, 1], f32, tag="hsbc", name="hsbc", bufs=2)
                    bcast_col(hs11[:], hsbc[:])

                    nc.scalar.activation(lh[:], lh[:], AF.Exp, scale=hsbc[:])
                    es = lh

                    pcx2 = ps.tile([D + 1, N], f32, tag="ps", name="ps")
                    for mt in range(MT):
                        nc.tensor.matmul(pcx2[:], vaug[mt][:, h * (D + 1):(h + 1) * (D + 1)],
                                         es[:, mt * N:(mt + 1) * N],
                                         start=(mt == 0), stop=(mt == MT - 1))
                    rcs = sm.tile([1, N], bf16, tag="rcs", name="rcs", bufs=2)
                    with nc.allow_low_precision(reason="bf16 reciprocal for bcast matmul"):
                        nc.vector.reciprocal(rcs[:], pcx2[D:D + 1, :])
                    prb = psum(D, N)
                    nc.tensor.matmul(prb[:], ones_row64[:], rcs[:], start=True, stop=True)
                    rcb = sm.tile([D, N], f32, tag="rcb", name="rcb", bufs=2)
                    nc.scalar.copy(rcb[:], prb[:])
                    nc.vector.tensor_tensor(ctxT[pt][off:off + D, :], pcx2[0:D, :],
                                            rcb[:], op=MULT)

                for nt2 in range(NT):
                    po = psum(P, E)
                    for pt in range(ET):
                        nc.tensor.matmul(po[:], ctxT[pt][:, nt2 * P:(nt2 + 1) * P],
                                         wo_sb[pt][:], start=(pt == 0), stop=(pt == ET - 1))
                    osb = scr.tile([P, E], bf16, tag="osb", name="osb", bufs=2)
                    nc.vector.tensor_copy(osb[:], po[:])
                    nc.sync.dma_start(out_d[br, nt2 * P:(nt2 + 1) * P, :], osb[:])

    nc.compile()
    return nc


def _get_nc():
    if "nc" not in _cache:
        _cache["nc"] = _build()
    return _cache["nc"]


def kernel(emb1, emb2, emb3, emb4, emb_C,
           Wq1, Wq2, Wq3, Wq4, Wk, Wv, WqC, WkC, WvC,
           Wo1, Wo2, Wo3, Wo4):
    from concourse.bass_utils import run_bass_kernel_spmd
    import os, time
    _tm = bool(os.environ.get("BASSK_TIMING"))
    _t0 = time.perf_counter()

    bf = ml_dtypes.bfloat16
    embs = [np.asarray(e, np.float32) for e in (emb1, emb2, emb3, emb4)]
    Wqs = [np.asarray(w, np.float32) for w in (Wq1, Wq2, Wq3, Wq4)]
    Wos = [np.asarray(w, np.float32) for w in (Wo1, Wo2, Wo3, Wo4)]
    emb_C = np.asarray(emb_C, np.float32)
    WqC, WkC, WvC = (np.asarray(w, np.float32) for w in (WqC, WkC, WvC))
    Wk, Wv = np.asarray(Wk, np.float32), np.asarray(Wv, np.float32)

    blob = np.empty((8, BLOB_ROWS, 1024), bf)

    # shared weight pack -> 8-way shards
    wpack = np.empty((WPACK_ROWS, 2048), bf)
    wpack[W_QC:W_QC + C] = WqC.T
    wpack[W_KC:W_KC + C] = WkC.T
    wpack[W_VC:W_VC + C] = WvC.T
    wpack[W_K:W_K + 128].reshape(E, E)[:] = Wk.T
    wpack[W_V:W_V + 128].reshape(E, E)[:] = Wv.T
    blob[:, R_WSH:BLOB_ROWS, :] = wpack.reshape(8, 1600, 1024)

    for b in range(B):
        ecT = emb_C[b].T.astype(bf)  # [2048, 512]
        for h in range(2):
            c = 4 * h + b
            blob[c, R_EMBC:R_E0, :].reshape(C, N)[:] = ecT
            blob[c, R_E0:R_E1, :].reshape(E, N)[:] = embs[2 * h][b].T
            blob[c, R_E1:R_WBR, :].reshape(E, N)[:] = embs[2 * h + 1][b].T
            # quad-gather contribution: rank b of [[0..3],[4..7]] ships
            # [Wq_2h, Wq_2h+1, Wo_2h, Wo_2h+1][b]
            wsrc = (Wqs[2 * h], Wqs[2 * h + 1], Wos[2 * h], Wos[2 * h + 1])[b]
            blob[c, R_WBR:R_WSH, :].reshape(E, E)[:] = wsrc.T

    in_maps = [{"blob": blob[c]} for c in range(8)]
    if _tm:
        print(f"[timing] host prep: {time.perf_counter()-_t0:.3f}s", file=sys.stderr)
        _t0 = time.perf_counter()

    nc = _get_nc()
    trace = bool(os.environ.get("BASSK_TRACE"))
    try:
        res = run_bass_kernel_spmd(nc, in_maps, core_ids=list(range(8)), trace=trace)
    except ModuleNotFoundError:
        # NTFF profile hook unavailable in this container; run untraced
        res = run_bass_kernel_spmd(nc, in_maps, core_ids=list(range(8)))
    _cache["last_result"] = res
    if _tm:
        print(f"[timing] run_bass_kernel_spmd: {time.perf_counter()-_t0:.3f}s", file=sys.stderr)
        _t0 = time.perf_counter()

    outs = []
    for br in range(4):
        h, j = br // 2, br % 2
        outs.append(np.stack([res.results[4 * h + b]["out"][j].astype(np.float32)
                              for b in range(B)]))
    if _tm:
        print(f"[timing] gather outputs: {time.perf_counter()-_t0:.3f}s", file=sys.stderr)
    return tuple(outs)


if __name__ == "__main__":
    sys.path.insert(0, "/root/problem")
    import reference
    inputs = reference.setup_inputs()
    inputs = {k: np.asarray(v) for k, v in inputs.items()}
    exp = reference.reference(**inputs)
    act = kernel(**inputs)
    for i, (a, e) in enumerate(zip(act, exp)):
        e = np.asarray(e)
        err = np.linalg.norm(a - e) / max(np.linalg.norm(e), 1e-30)
        print(f"out{i + 1}: rel_err={err:.3e}")


# revision 8
# speedup vs baseline: 1.1248x; 1.0933x over previous
"""Trainium2 Bass kernel for nn_Attention_65644280152570.

Dual attention: channel cross-attention (C=2048) produces shared K/V tokens
for 4 spatial multi-head (H=8) cross-attention branches.

Sharding (8 cores): core = 4*h + b with b=batch, h=branch-pair. Each core
computes the full channel branch for its batch (replicated between the two
cores sharing a batch) plus 2 of the 4 spatial branches.

Wire-optimized input distribution (the axon host->device tunnel is ~66MB/s,
so shipped bytes dominate wall time): every input byte is shipped exactly
once where possible. Each core receives ONE packed bf16 blob [3392, 1024]:
  rows [   0,1024): emb_C[b]^T  [2048x512]  (private; x2 across the pair)
  rows [1024,1280): emb_{2h}[b]^T   [512x512] (private, x1)
  rows [1280,1536): emb_{2h+1}[b]^T [512x512] (private, x1)
  rows [1536,1792): branch-weight contribution [512x512] (quad-AllGather, x1)
  rows [1792,3392): 1/8 shard of the shared weight pack (8-way AllGather, x1)
The weight pack [6400, 2048] holds WqC^T, WkC^T, WvC^T, Wk^T, Wv^T; the quad
gather over [[0,1,2,3],[4,5,6,7]] assembles [Wq_{2h}^T, Wq_{2h+1}^T,
Wo_{2h}^T, Wo_{2h+1}^T] (rank r=batch contributes tensor r, so every core
reads the gathered buffer at fixed offsets). Outputs are bf16 (halves d2h).

All matmuls bf16 with f32 PSUM accumulation.

Softmax trick: softmax(inorm(x)) == softmax(x * rsqrt(var(x)+eps)) (the mean
shift cancels row-wise), and logits are ~N(0,1) after scaling so no max
subtraction is needed. Attention maps are kept transposed ([keys, queries])
so the softmax axis sits on partitions and feeds the context matmul
contraction directly; column sums come from ones-augmented matmuls.
"""

import sys
import numpy as np

for p in ("/opt/trn_rl_repo", "/root/.axon_site/_ro/trn_rl_repo"):
    if p not in sys.path:
        sys.path.insert(0, p)

import ml_dtypes

B, N, E, H = 4, 512, 512, 8
C = 4 * E          # 2048
D = E // H         # 64
P = 128
NT = N // P        # 4 n-tiles
CT = C // P        # 16 c/d tiles
ET = E // P        # 4 e-tiles
MT = (4 * N) // P  # 16 token tiles
EPS = 1e-5
M_CH = float(C * C)        # channel inorm element count
M_SP = float(N * 4 * N)    # spatial inorm element count per head

# blob geometry (bf16, width 1024); row offsets of each section
R_EMBC = 0          # 1024 rows: embcT [2048, 512]
R_E0 = 1024         # 256 rows: e0T [512, 512]
R_E1 = 1280         # 256 rows: e1T [512, 512]
R_WBR = 1536        # 256 rows: quad-gather contribution [512, 512]
R_WSH = 1792        # 1600 rows: weight-pack shard [800, 2048]
BLOB_ROWS = 3392
# weight pack row offsets (width 2048)
W_QC = 0            # [2048, 2048]
W_KC = 2048
W_VC = 4096
W_K = 6144          # [512, 512] stored as [128, 2048]
W_V = 6272
WPACK_ROWS = 6400

BF16 = "bfloat16"
_cache = {}


def _build():
    import concourse.bass as bass
    import concourse.mybir as mybir
    import concourse.tile as tile
    from concourse import bacc

    f32 = mybir.dt.float32
    bf16 = mybir.dt.bfloat16
    AX = mybir.AxisListType.X
    ADD = mybir.AluOpType.add
    MULT = mybir.AluOpType.mult
    SUB = mybir.AluOpType.subtract
    BYP = mybir.AluOpType.bypass
    AF = mybir.ActivationFunctionType

    nc = bacc.Bacc("TRN2", target_bir_lowering=False, debug=False, num_devices=8)

    blob_d = nc.dram_tensor("blob", [BLOB_ROWS, 1024], bf16, kind="ExternalInput")
    out_d = nc.dram_tensor("out", [2, N, E], bf16, kind="ExternalOutput")

    def half(sl_rows, b=2):
        # [r, 1024]-rowspace view -> [(r*b), 2048//b] logical rows
        return sl_rows.rearrange("a (b c) -> (a b) c", b=b)

    with tile.TileContext(nc) as tc:
        import contextlib
        ctx = contextlib.ExitStack()
        with ctx:
            const = ctx.enter_context(tc.tile_pool(name="const", bufs=1))
            wpool = ctx.enter_context(tc.tile_pool(name="wpool", bufs=1))
            ps = ctx.enter_context(tc.tile_pool(name="ps", bufs=8, space="PSUM"))
            big = ctx.enter_context(tc.tile_pool(name="big", bufs=1))
            sm = ctx.enter_context(tc.tile_pool(name="sm", bufs=1))
            scr = ctx.enter_context(tc.tile_pool(name="scr", bufs=1))
            dram = ctx.enter_context(tc.tile_pool(name="dram", bufs=2, space="DRAM"))

            # ---------------- collective prologue: distribute weights ------
            wsh_i = dram.tile([800, 2048], bf16, tag="wshi", name="wshi")
            wpg = dram.tile([WPACK_ROWS, 2048], bf16, tag="wpg", name="wpg",
                            addr_space="Shared")
            wbr_i = dram.tile([E, E], bf16, tag="wbri", name="wbri")
            wbrg = dram.tile([4 * E, E], bf16, tag="wbrg", name="wbrg")
            nc.gpsimd.dma_start(wsh_i[:], half(blob_d[R_WSH:BLOB_ROWS, :]))
            nc.gpsimd.dma_start(wbr_i[:], half(blob_d[R_WBR:R_WSH, :]))
            nc.gpsimd.collective_compute(
                "AllGather", BYP, replica_groups=[list(range(8))],
                ins=[wsh_i.opt()], outs=[wpg.opt()])
            nc.gpsimd.collective_compute(
                "AllGather", BYP, replica_groups=[[0, 1, 2, 3], [4, 5, 6, 7]],
                ins=[wbr_i.opt()], outs=[wbrg.opt()])

            ones_col = const.tile([P, 1], bf16, tag="oc", name="oc")
            nc.any.memset(ones_col[:], 1.0)
            ones_col_f = const.tile([P, 1], f32, tag="ocf", name="ocf")
            nc.any.memset(ones_col_f[:], 1.0)
            ones_row_f = const.tile([1, P], f32, tag="orf", name="orf")
            nc.any.memset(ones_row_f[:], 1.0)
            ones_row64 = const.tile([1, D], bf16, tag="or64", name="or64")
            nc.any.memset(ones_row64[:], 1.0)
            eps11 = const.tile([1, 1], f32, tag="eps11", name="eps11")
            nc.any.memset(eps11[:], EPS)

            def psum(p_, n_):
                return ps.tile([p_, n_], f32, tag="ps", name="ps")

            # f32 cross-partition sum: [128,1] f32 -> [1,1] f32 in psum, evict
            def part_sum(src_col, out11):
                pt = psum(1, 1)
                nc.tensor.matmul(pt[:], ones_col_f[:], src_col, start=True, stop=True)
                nc.scalar.copy(out11, pt[:])

            # broadcast [1,1] f32 -> [128,1] f32 (K=1 matmul)
            def bcast_col(src11, out_col):
                pt = psum(P, 1)
                nc.tensor.matmul(pt[:], ones_row_f[:], src11, start=True, stop=True)
                nc.scalar.copy(out_col, pt[:])

            # ---------------- stage A: load embcT, compute QC, KC, VCT ----
            embcT = [big.tile([P, N], bf16, tag="embva", name="embcT", bufs=16, padded_shape=[P, 528]) for _ in range(CT)]
            for kt in range(CT):
                nc.sync.dma_start(embcT[kt][:], half(blob_d[R_EMBC + kt * 64:R_EMBC + (kt + 1) * 64, :]))

            qc = [big.tile([P, C], bf16, tag="qc", name="qc", bufs=4) for _ in range(NT)]
            kc = [big.tile([P, C], bf16, tag="kc", name="kc", bufs=4) for _ in range(NT)]
            for woff, dst in ((W_QC, qc), (W_KC, kc)):
                for ch in range(4):
                    pts = [psum(P, 512) for _ in range(NT)]
                    for kt in range(CT):
                        wt = wpool.tile([P, 512], bf16, tag="wck", name="wck", bufs=3)
                        nc.sync.dma_start(wt[:], wpg[woff + kt * P:woff + (kt + 1) * P, ch * 512:(ch + 1) * 512])
                        for nt in range(NT):
                            nc.tensor.matmul(pts[nt][:], embcT[kt][:, nt * P:(nt + 1) * P],
                                             wt[:], start=(kt == 0), stop=(kt == CT - 1))
                    for nt in range(NT):
                        nc.vector.tensor_copy(dst[nt][:, ch * 512:(ch + 1) * 512], pts[nt][:])

            vct = [big.tile([P, N], bf16, tag="vct", name="vct", bufs=16) for _ in range(CT)]
            for dtg in range(4):
                pts = [psum(P, N) for _ in range(4)]
                for kt in range(CT):
                    wt = wpool.tile([P, 512], bf16, tag="wvk", name="wvk", bufs=3)
                    nc.sync.dma_start(wt[:], wpg[W_VC + kt * P:W_VC + (kt + 1) * P, dtg * 512:(dtg + 1) * 512])
                    for q in range(4):
                        nc.tensor.matmul(pts[q][:], wt[:, q * P:(q + 1) * P], embcT[kt][:],
                                         start=(kt == 0), stop=(kt == CT - 1))
                for q in range(4):
                    nc.vector.tensor_copy(vct[dtg * 4 + q][:], pts[q][:])

            # ---------------- channel attention: A' = attn^T [d, c] -------
            # A' chunks -> DRAM (SBUF can't hold 16MB of A' and E'); global
            # stats accumulate on the fly.
            apd = dram.tile([C, C], bf16, tag="apd", name="apd")
            epd = dram.tile([C, C], bf16, tag="epd", name="epd")
            smsl = sm.tile([P, 64], f32, tag="smsl", name="smsl")
            sqsl = sm.tile([P, 64], f32, tag="sqsl", name="sqsl")
            for dt in range(CT):
                for ch in range(4):
                    pa = psum(P, 512)
                    for nt in range(NT):
                        nc.tensor.matmul(pa[:], kc[nt][:, dt * P:(dt + 1) * P],
                                         qc[nt][:, ch * 512:(ch + 1) * 512],
                                         start=(nt == 0), stop=(nt == NT - 1))
                    idx = dt * 4 + ch
                    sqs = scr.tile([P, 512], bf16, tag="sqs", name="sqs", bufs=2)
                    nc.scalar.activation(sqs[:], pa[:], AF.Square,
                                         accum_out=sqsl[:, idx:idx + 1])
                    apw = scr.tile([P, 512], bf16, tag="apw", name="apw", bufs=3)
                    with nc.allow_low_precision(reason="bf16 evict, f32 accum"):
                        nc.vector.tensor_scalar(apw[:], pa[:], 0.0, 0.0, op0=ADD, op1=ADD,
                                                accum_out=smsl[:, idx:idx + 1])
                    nc.sync.dma_start(apd[dt * P:(dt + 1) * P, ch * 512:(ch + 1) * 512], apw[:])

            # stats -> scale s = 1/sqrt(var+eps), broadcast to [128,1]
            smv = sm.tile([P, 1], f32, tag="smv", name="smv")
            sqv = sm.tile([P, 1], f32, tag="sqv", name="sqv")
            nc.vector.tensor_reduce(smv[:], smsl[:], AX, ADD)
            nc.vector.tensor_reduce(sqv[:], sqsl[:], AX, ADD)
            stot = sm.tile([1, 1], f32, tag="stot", name="stot")
            qtot = sm.tile([1, 1], f32, tag="qtot", name="qtot")
            part_sum(smv[:], stot[:])
            part_sum(sqv[:], qtot[:])
            m2 = sm.tile([1, 1], f32, tag="m2", name="m2")
            t2 = sm.tile([1, 1], f32, tag="t2", name="t2")
            nc.scalar.activation(m2[:], stot[:], AF.Square, scale=1.0 / M_CH)
            nc.scalar.activation(t2[:], qtot[:], AF.Copy, scale=1.0 / M_CH)
            var1 = sm.tile([1, 1], f32, tag="var1", name="var1")
            nc.vector.tensor_tensor(var1[:], t2[:], m2[:], op=SUB)
            sd1 = sm.tile([1, 1], f32, tag="sd1", name="sd1")
            nc.scalar.activation(sd1[:], var1[:], AF.Sqrt, bias=eps11[:])
            s11 = sm.tile([1, 1], f32, tag="s11", name="s11")
            nc.vector.reciprocal(s11[:], sd1[:])
            sbc = sm.tile([P, 1], f32, tag="sbc", name="sbc")
            bcast_col(s11[:], sbc[:])

            # pass A: stream A' from DRAM, exp, accumulate column sums over
            # d (partitions, via ones-lhsT matmul); write E' back to DRAM
            pcs = [psum(1, 512) for _ in range(4)]
            for dt in range(CT):
                apr = scr.tile([P, C], bf16, tag="apr", name="apr", bufs=3)
                nc.sync.dma_start(apr[:], apd[dt * P:(dt + 1) * P, :])
                nc.scalar.activation(apr[:], apr[:], AF.Exp, scale=sbc[:])
                for ch in range(4):
                    nc.tensor.matmul(pcs[ch][:], ones_col[:],
                                     apr[:, ch * 512:(ch + 1) * 512],
                                     start=(dt == 0), stop=(dt == CT - 1))
                nc.sync.dma_start(epd[dt * P:(dt + 1) * P, :], apr[:])
            rr = sm.tile([1, C], f32, tag="rr", name="rr")
            for ch in range(4):
                nc.vector.reciprocal(rr[:, ch * 512:(ch + 1) * 512], pcs[ch][:])
            # transpose [1, C] -> [128, 16] via DRAM bounce
            rb_d = dram.tile([1, C], f32, tag="rb", name="rb")
            nc.sync.dma_start(rb_d[:], rr[:])
            rT = sm.tile([P, CT], f32, tag="rT", name="rT")
            nc.sync.dma_start(rT[:], rb_d[:].rearrange("a (t p) -> (a p) t", p=P))

            # pass B: ctx[c,n] = (E'^T @ VCT) * recip_colsum[c], two groups of
            # 8 PSUM accumulators; E' streamed per d-tile
            ctx_sb = [big.tile([P, N], bf16, tag="ctx", name="ctx", bufs=16) for _ in range(CT)]
            for g in range(2):
                pcxs = [psum(P, N) for _ in range(8)]
                for dt in range(CT):
                    epr = scr.tile([P, C], bf16, tag="apr", name="epr", bufs=3)
                    nc.sync.dma_start(epr[:], epd[dt * P:(dt + 1) * P, :])
                    for k in range(8):
                        ct = g * 8 + k
                        nc.tensor.matmul(pcxs[k][:], epr[:, ct * P:(ct + 1) * P], vct[dt][:],
                                         start=(dt == 0), stop=(dt == CT - 1))
                for k in range(8):
                    ct = g * 8 + k
                    nc.vector.tensor_scalar_mul(ctx_sb[ct][:], pcxs[k][:], rT[:, ct:ct + 1])

            # ---------------- shared K/V over the 4N gathered tokens ------
            wk_sb = [sm.tile([P, E], bf16, tag="wk", name="wk", bufs=4) for _ in range(ET)]
            wv_sb = [sm.tile([P, E], bf16, tag="wv", name="wv", bufs=4) for _ in range(ET)]
            for et in range(ET):
                nc.sync.dma_start(wk_sb[et][:], half(wpg[W_K + et * 32:W_K + (et + 1) * 32, :], b=4))
                nc.sync.dma_start(wv_sb[et][:], half(wpg[W_V + et * 32:W_V + (et + 1) * 32, :], b=4))

            kt_sb = [big.tile([P, 4 * N], bf16, tag="kt", name="kt", bufs=4) for _ in range(ET)]
            for pt in range(ET):
                for j in range(4):
                    pk = psum(P, 512)
                    for et in range(ET):
                        nc.tensor.matmul(pk[:], wk_sb[et][:, pt * P:(pt + 1) * P],
                                         ctx_sb[4 * j + et][:],
                                         start=(et == 0), stop=(et == ET - 1))
                    nc.vector.tensor_copy(kt_sb[pt][:, j * 512:(j + 1) * 512], pk[:])

            vaug = [big.tile([P, H * (D + 1)], bf16, tag="embva", name="vaug", bufs=16, padded_shape=[P, 528]) for _ in range(MT)]
            for mt in range(MT):
                j, q = mt // 4, mt % 4
                pv = psum(P, 512)
                for et in range(ET):
                    nc.tensor.matmul(pv[:], ctx_sb[4 * j + et][:, q * P:(q + 1) * P],
                                     wv_sb[et][:], start=(et == 0), stop=(et == ET - 1))
                va = vaug[mt][:].rearrange("p (h x) -> p h x", x=D + 1)
                nc.vector.tensor_copy(va[:, :, 0:D], pv[:].rearrange("p (h x) -> p h x", x=D))
                nc.any.memset(va[:, :, D:D + 1], 1.0)

            # ---------------- two spatial branches -------------------------
            for br in range(2):
                ebT = [sm.tile([P, N], bf16, tag="ebT", name="ebT", bufs=4) for _ in range(ET)]
                wq_sb = [sm.tile([P, E], bf16, tag="wq", name="wq", bufs=4) for _ in range(ET)]
                wo_sb = [sm.tile([P, E], bf16, tag="wo", name="wo", bufs=4) for _ in range(ET)]
                r_e = R_E0 if br == 0 else R_E1
                for et in range(ET):
                    nc.sync.dma_start(ebT[et][:], half(blob_d[r_e + et * 64:r_e + (et + 1) * 64, :]))
                    nc.sync.dma_start(wq_sb[et][:], wbrg[br * 512 + et * P:br * 512 + (et + 1) * P, :])
                    nc.sync.dma_start(wo_sb[et][:], wbrg[1024 + br * 512 + et * P:1024 + br * 512 + (et + 1) * P, :])

                qt_sb = [sm.tile([P, N], bf16, tag="qt", name="qt", bufs=4) for _ in range(ET)]
                for pt in range(ET):
                    pq = psum(P, N)
                    for et in range(ET):
                        nc.tensor.matmul(pq[:], wq_sb[et][:, pt * P:(pt + 1) * P],
                                         ebT[et][:], start=(et == 0), stop=(et == ET - 1))
                    nc.vector.tensor_copy(qt_sb[pt][:], pq[:])

                ctxT = [sm.tile([P, N], bf16, tag="ctxT", name="ctxT", bufs=8) for _ in range(ET)]
                for h in range(H):
                    pt, off = h // 2, (h % 2) * D
                    lh = big.tile([P, MT * N], bf16, tag="lh", name="lh", bufs=2)
                    hsm = sm.tile([P, MT], f32, tag="hsm", name="hsm", bufs=2)
                    hsq = sm.tile([P, MT], f32, tag="hsq", name="hsq", bufs=2)
                    for mt in range(MT):
                        pl = psum(P, N)
                        nc.tensor.matmul(pl[:], kt_sb[pt][off:off + D, mt * P:(mt + 1) * P],
                                         qt_sb[pt][off:off + D, :], start=True, stop=True)
                        sqs = scr.tile([P, 512], bf16, tag="sqs", name="sqs", bufs=2)
                        nc.scalar.activation(sqs[:], pl[:], AF.Square,
                                             accum_out=hsq[:, mt:mt + 1])
                        with nc.allow_low_precision(reason="bf16 evict, f32 accum"):
                            nc.vector.tensor_scalar(lh[:, mt * N:(mt + 1) * N], pl[:],
                                                    0.0, 0.0, op0=ADD, op1=ADD,
                                                    accum_out=hsm[:, mt:mt + 1])
                    hsmv = sm.tile([P, 1], f32, tag="hsmv", name="hsmv", bufs=2)
                    hsqv = sm.tile([P, 1], f32, tag="hsqv", name="hsqv", bufs=2)
                    nc.vector.tensor_reduce(hsmv[:], hsm[:], AX, ADD)
                    nc.vector.tensor_reduce(hsqv[:], hsq[:], AX, ADD)
                    hst = sm.tile([1, 1], f32, tag="hst", name="hst", bufs=2)
                    hqt = sm.tile([1, 1], f32, tag="hqt", name="hqt", bufs=2)
                    part_sum(hsmv[:], hst[:])
                    part_sum(hsqv[:], hqt[:])
                    hm2 = sm.tile([1, 1], f32, tag="hm2", name="hm2", bufs=2)
                    ht2 = sm.tile([1, 1], f32, tag="ht2", name="ht2", bufs=2)
                    nc.scalar.activation(hm2[:], hst[:], AF.Square, scale=1.0 / M_SP)
                    nc.scalar.activation(ht2[:], hqt[:], AF.Copy, scale=1.0 / M_SP)
                    hvar = sm.tile([1, 1], f32, tag="hvar", name="hvar", bufs=2)
                    nc.vector.tensor_tensor(hvar[:], ht2[:], hm2[:], op=SUB)
                    hsd1 = sm.tile([1, 1], f32, tag="hsd1", name="hsd1", bufs=2)
                    nc.scalar.activation(hsd1[:], hvar[:], AF.Sqrt, bias=eps11[:])
                    hs11 = sm.tile([1, 1], f32, tag="hs11", name="hs11", bufs=2)
                    nc.vector.reciprocal(hs11[:], hsd1[:])
                    hsbc = sm.tile([P, 1], f32, tag="hsbc", name="hsbc", bufs=2)
                    bcast_col(hs11[:], hsbc[:])

                    nc.scalar.activation(lh[:], lh[:], AF.Exp, scale=hsbc[:])
                    es = lh

                    pcx2 = ps.tile([D + 1, N], f32, tag="ps", name="ps")
                    for mt in range(MT):
                        nc.tensor.matmul(pcx2[:], vaug[mt][:, h * (D + 1):(h + 1) * (D + 1)],
                                         es[:, mt * N:(mt + 1) * N],
                                         start=(mt == 0), stop=(mt == MT - 1))
                    rcs = sm.tile([1, N], bf16, tag="rcs", name="rcs", bufs=2)
                    with nc.allow_low_precision(reason="bf16 reciprocal for bcast matmul"):
                        nc.vector.reciprocal(rcs[:], pcx2[D:D + 1, :])
                    prb = psum(D, N)
                    nc.tensor.matmul(prb[:], ones_row64[:], rcs[:], start=True, stop=True)
                    rcb = sm.tile([D, N], f32, tag="rcb", name="rcb", bufs=2)
                    nc.scalar.copy(rcb[:], prb[:])
                    nc.vector.tensor_tensor(ctxT[pt][off:off + D, :], pcx2[0:D, :],
                                            rcb[:], op=MULT)

                for nt2 in range(NT):
                    po = psum(P, E)
                    for pt in range(ET):
                        nc.tensor.matmul(po[:], ctxT[pt][:, nt2 * P:(nt2 + 1) * P],
                                         wo_sb[pt][:], start=(pt == 0), stop=(pt == ET - 1))
                    osb = scr.tile([P, E], bf16, tag="osb", name="osb", bufs=2)
                    nc.vector.tensor_copy(osb[:], po[:])
                    nc.sync.dma_start(out_d[br, nt2 * P:(nt2 + 1) * P, :], osb[:])

    nc.compile()
    return nc


def _get_nc():
    if "nc" not in _cache:
        _cache["nc"] = _build()
    return _cache["nc"]


def kernel(emb1, emb2, emb3, emb4, emb_C,
           Wq1, Wq2, Wq3, Wq4, Wk, Wv, WqC, WkC, WvC,
           Wo1, Wo2, Wo3, Wo4):
    from concourse.bass_utils import run_bass_kernel_spmd
    import os, time
    _tm = bool(os.environ.get("BASSK_TIMING"))
    _t0 = time.perf_counter()

    bf = ml_dtypes.bfloat16
    embs = [np.asarray(e, np.float32) for e in (emb1, emb2, emb3, emb4)]
    Wqs = [np.asarray(w, np.float32) for w in (Wq1, Wq2, Wq3, Wq4)]
    Wos = [np.asarray(w, np.float32) for w in (Wo1, Wo2, Wo3, Wo4)]
    emb_C = np.asarray(emb_C, np.float32)
    WqC, WkC, WvC = (np.asarray(w, np.float32) for w in (WqC, WkC, WvC))
    Wk, Wv = np.asarray(Wk, np.float32), np.asarray(Wv, np.float32)

    blob = np.empty((8, BLOB_ROWS, 1024), bf)

    # shared weight pack -> 8-way shards
    # NB: two-step (.T.astype then bf16 memcpy) — a direct strided-cast
    # assignment into bf16 goes through numpy's slow buffered-iterator path
    wpack = np.empty((WPACK_ROWS, 2048), bf)
    wpack[W_QC:W_QC + C] = WqC.T.astype(bf)
    wpack[W_KC:W_KC + C] = WkC.T.astype(bf)
    wpack[W_VC:W_VC + C] = WvC.T.astype(bf)
    wpack[W_K:W_K + 128].reshape(E, E)[:] = Wk.T.astype(bf)
    wpack[W_V:W_V + 128].reshape(E, E)[:] = Wv.T.astype(bf)
    blob[:, R_WSH:BLOB_ROWS, :] = wpack.reshape(8, 1600, 1024)

    for b in range(B):
        ecT = emb_C[b].T.astype(bf)  # [2048, 512]
        for h in range(2):
            c = 4 * h + b
            blob[c, R_EMBC:R_E0, :].reshape(C, N)[:] = ecT
            blob[c, R_E0:R_E1, :].reshape(E, N)[:] = embs[2 * h][b].T.astype(bf)
            blob[c, R_E1:R_WBR, :].reshape(E, N)[:] = embs[2 * h + 1][b].T.astype(bf)
            # quad-gather contribution: rank b of [[0..3],[4..7]] ships
            # [Wq_2h, Wq_2h+1, Wo_2h, Wo_2h+1][b]
            wsrc = (Wqs[2 * h], Wqs[2 * h + 1], Wos[2 * h], Wos[2 * h + 1])[b]
            blob[c, R_WBR:R_WSH, :].reshape(E, E)[:] = wsrc.T.astype(bf)

    in_maps = [{"blob": blob[c]} for c in range(8)]
    if _tm:
        print(f"[timing] host prep: {time.perf_counter()-_t0:.3f}s", file=sys.stderr)
        _t0 = time.perf_counter()

    nc = _get_nc()
    trace = bool(os.environ.get("BASSK_TRACE"))
    try:
        res = run_bass_kernel_spmd(nc, in_maps, core_ids=list(range(8)), trace=trace)
    except ModuleNotFoundError:
        # NTFF profile hook unavailable in this container; run untraced
        res = run_bass_kernel_spmd(nc, in_maps, core_ids=list(range(8)))
    _cache["last_result"] = res
    if _tm:
        print(f"[timing] run_bass_kernel_spmd: {time.perf_counter()-_t0:.3f}s", file=sys.stderr)
        _t0 = time.perf_counter()

    outs = []
    for br in range(4):
        h, j = br // 2, br % 2
        outs.append(np.stack([res.results[4 * h + b]["out"][j].astype(np.float32)
                              for b in range(B)]))
    if _tm:
        print(f"[timing] gather outputs: {time.perf_counter()-_t0:.3f}s", file=sys.stderr)
    return tuple(outs)


if __name__ == "__main__":
    sys.path.insert(0, "/root/problem")
    import reference
    inputs = reference.setup_inputs()
    inputs = {k: np.asarray(v) for k, v in inputs.items()}
    exp = reference.reference(**inputs)
    act = kernel(**inputs)
    for i, (a, e) in enumerate(zip(act, exp)):
        e = np.asarray(e)
        err = np.linalg.norm(a - e) / max(np.linalg.norm(e), 1e-30)
        print(f"out{i + 1}: rel_err={err:.3e}")


# revision 15
# speedup vs baseline: 1.1989x; 1.0658x over previous
"""Trainium2 Bass kernel for nn_Attention_65644280152570.

Dual attention: channel cross-attention (C=2048) produces shared K/V tokens
for 4 spatial multi-head (H=8) cross-attention branches.

Sharding (8 cores): core = 4*h + b with b=batch, h=branch-pair. Each core
computes the full channel branch for its batch (replicated between the two
cores sharing a batch) plus 2 of the 4 spatial branches.

Wire-optimized input distribution (the axon host->device tunnel is ~66MB/s,
so shipped bytes dominate wall time): every input byte is shipped exactly
once where possible. Each core receives ONE packed bf16 blob [3392, 1024]:
  rows [   0,1024): emb_C[b]^T  [2048x512]  (private; x2 across the pair)
  rows [1024,1280): emb_{2h}[b]^T   [512x512] (private, x1)
  rows [1280,1536): emb_{2h+1}[b]^T [512x512] (private, x1)
  rows [1536,1792): branch-weight contribution [512x512] (quad-AllGather, x1)
  rows [1792,3392): 1/8 shard of the shared weight pack (8-way AllGather, x1)
The weight pack [6400, 2048] holds WqC^T, WkC^T, WvC^T, Wk^T, Wv^T; the quad
gather over [[0,1,2,3],[4,5,6,7]] assembles [Wq_{2h}^T, Wq_{2h+1}^T,
Wo_{2h}^T, Wo_{2h+1}^T] (rank r=batch contributes tensor r, so every core
reads the gathered buffer at fixed offsets). Outputs are bf16 (halves d2h).

All matmuls bf16 with f32 PSUM accumulation.

Softmax trick: softmax(inorm(x)) == softmax(x * rsqrt(var(x)+eps)) (the mean
shift cancels row-wise), and logits are ~N(0,1) after scaling so no max
subtraction is needed. Attention maps are kept transposed ([keys, queries])
so the softmax axis sits on partitions and feeds the context matmul
contraction directly; column sums come from ones-augmented matmuls.
"""

import sys
import numpy as np

for p in ("/opt/trn_rl_repo", "/root/.axon_site/_ro/trn_rl_repo"):
    if p not in sys.path:
        sys.path.insert(0, p)

import ml_dtypes

B, N, E, H = 4, 512, 512, 8
C = 4 * E          # 2048
D = E // H         # 64
P = 128
NT = N // P        # 4 n-tiles
CT = C // P        # 16 c/d tiles
ET = E // P        # 4 e-tiles
MT = (4 * N) // P  # 16 token tiles
EPS = 1e-5
M_CH = float(C * C)        # channel inorm element count
M_SP = float(N * 4 * N)    # spatial inorm element count per head

# blob geometry (bf16, width 1024); row offsets of each section
R_EMBH = 0          # 512 rows: embC carry-half [1024, 512] (8-way gather, x1)
R_E0 = 512          # 256 rows: e0T [512, 512]
R_E1 = 768          # 256 rows: e1T [512, 512]
R_WBR = 1024        # 256 rows: quad-gather contribution [512, 512]
R_WSH = 1280        # 1600 rows: weight-pack shard [800, 2048]
R_SEL = 2880        # 1 row: cols 0:4 = one-hot(batch) selector
BLOB_ROWS = 2881
# weight pack row offsets (width 2048)
W_QC = 0            # [2048, 2048]
W_KC = 2048
W_VC = 4096
W_K = 6144          # [512, 512] stored as [128, 2048]
W_V = 6272
WPACK_ROWS = 6400

BF16 = "bfloat16"
_cache = {}


def _build():
    import concourse.bass as bass
    import concourse.mybir as mybir
    import concourse.tile as tile
    from concourse import bacc

    f32 = mybir.dt.float32
    bf16 = mybir.dt.bfloat16
    AX = mybir.AxisListType.X
    ADD = mybir.AluOpType.add
    MULT = mybir.AluOpType.mult
    SUB = mybir.AluOpType.subtract
    BYP = mybir.AluOpType.bypass
    AF = mybir.ActivationFunctionType

    nc = bacc.Bacc("TRN2", target_bir_lowering=False, debug=False, num_devices=8)

    blob_d = nc.dram_tensor("blob", [BLOB_ROWS, 1024], bf16, kind="ExternalInput")
    out_d = nc.dram_tensor("out", [2, N, E], bf16, kind="ExternalOutput")

    def half(sl_rows, b=2):
        # [r, 1024]-rowspace view -> [(r*b), 2048//b] logical rows
        return sl_rows.rearrange("a (b c) -> (a b) c", b=b)

    with tile.TileContext(nc) as tc:
        import contextlib
        ctx = contextlib.ExitStack()
        with ctx:
            const = ctx.enter_context(tc.tile_pool(name="const", bufs=1))
            wpool = ctx.enter_context(tc.tile_pool(name="wpool", bufs=1))
            ps = ctx.enter_context(tc.tile_pool(name="ps", bufs=8, space="PSUM"))
            big = ctx.enter_context(tc.tile_pool(name="big", bufs=1))
            sm = ctx.enter_context(tc.tile_pool(name="sm", bufs=1))
            scr = ctx.enter_context(tc.tile_pool(name="scr", bufs=1))
            dram = ctx.enter_context(tc.tile_pool(name="dram", bufs=2, space="DRAM"))

            # ---------------- collective prologue: distribute weights ------
            wsh_i = dram.tile([800, 2048], bf16, tag="wshi", name="wshi")
            wpg = dram.tile([WPACK_ROWS, 2048], bf16, tag="wpg", name="wpg",
                            addr_space="Shared")
            wbr_i = dram.tile([E, E], bf16, tag="wbri", name="wbri")
            wbrg = dram.tile([4 * E, E], bf16, tag="wbrg", name="wbrg")
            embc_i = dram.tile([1024, N], bf16, tag="embci", name="embci")
            embA = dram.tile([4 * C, N], bf16, tag="embA", name="embA",
                             addr_space="Shared")
            nc.gpsimd.dma_start(wsh_i[:], half(blob_d[R_WSH:R_SEL, :]))
            nc.gpsimd.dma_start(wbr_i[:], half(blob_d[R_WBR:R_WSH, :]))
            nc.gpsimd.dma_start(embc_i[:], half(blob_d[R_EMBH:R_E0, :]))
            nc.gpsimd.collective_compute(
                "AllGather", BYP, replica_groups=[list(range(8))],
                ins=[embc_i.opt()], outs=[embA.opt()])
            nc.gpsimd.collective_compute(
                "AllGather", BYP, replica_groups=[list(range(8))],
                ins=[wsh_i.opt()], outs=[wpg.opt()])
            nc.gpsimd.collective_compute(
                "AllGather", BYP, replica_groups=[[0, 1, 2, 3], [4, 5, 6, 7]],
                ins=[wbr_i.opt()], outs=[wbrg.opt()])

            ones_col = const.tile([P, 1], bf16, tag="oc", name="oc")
            nc.any.memset(ones_col[:], 1.0)
            ones_col_f = const.tile([P, 1], f32, tag="ocf", name="ocf")
            nc.any.memset(ones_col_f[:], 1.0)
            ones_row_f = const.tile([1, P], f32, tag="orf", name="orf")
            nc.any.memset(ones_row_f[:], 1.0)
            ones_row64 = const.tile([1, D], bf16, tag="or64", name="or64")
            nc.any.memset(ones_row64[:], 1.0)
            eps11 = const.tile([1, 1], f32, tag="eps11", name="eps11")
            nc.any.memset(eps11[:], EPS)

            def psum(p_, n_):
                return ps.tile([p_, n_], f32, tag="ps", name="ps")

            # f32 cross-partition sum: [128,1] f32 -> [1,1] f32 in psum, evict
            def part_sum(src_col, out11):
                pt = psum(1, 1)
                nc.tensor.matmul(pt[:], ones_col_f[:], src_col, start=True, stop=True)
                nc.scalar.copy(out11, pt[:])

            # broadcast [1,1] f32 -> [128,1] f32 (K=1 matmul)
            def bcast_col(src11, out_col):
                pt = psum(P, 1)
                nc.tensor.matmul(pt[:], ones_row_f[:], src11, start=True, stop=True)
                nc.scalar.copy(out_col, pt[:])

            # batch one-hot selector -> four [P,1] f32 broadcast columns
            sel_sb = sm.tile([1, 4], bf16, tag="selb", name="selb")
            nc.sync.dma_start(sel_sb[:], blob_d[R_SEL:R_SEL + 1, 0:4])
            sel_f = sm.tile([1, 4], f32, tag="self", name="self")
            nc.scalar.copy(sel_f[:], sel_sb[:])
            selc = sm.tile([P, 4], f32, tag="selc", name="selc")
            for b2 in range(4):
                bcast_col(sel_f[:, b2:b2 + 1], selc[:, b2:b2 + 1])

            # ---------------- stage A: reconstruct embcT (one-hot over the
            # gathered all-batch embA; exact since weights are 0/1), then
            # compute QC, KC, VCT
            embcT = [big.tile([P, N], bf16, tag="embva", name="embcT", bufs=16, padded_shape=[P, 528]) for _ in range(CT)]
            for kt in range(CT):
                srcs = [scr.tile([P, N], bf16, tag="ecs", name="ecs", bufs=4) for _ in range(4)]
                for b2 in range(4):
                    nc.sync.dma_start(srcs[b2][:], embA[b2 * C + kt * P:b2 * C + (kt + 1) * P, :])
                nc.vector.tensor_scalar_mul(embcT[kt][:], srcs[0][:], selc[:, 0:1])
                for b2 in range(1, 4):
                    tmp = scr.tile([P, N], bf16, tag="ect", name="ect", bufs=1)
                    nc.vector.tensor_scalar_mul(tmp[:], srcs[b2][:], selc[:, b2:b2 + 1])
                    nc.vector.tensor_tensor(embcT[kt][:], embcT[kt][:], tmp[:], op=ADD)

            qc = [big.tile([P, C], bf16, tag="qc", name="qc", bufs=4) for _ in range(NT)]
            kc = [big.tile([P, C], bf16, tag="kc", name="kc", bufs=4) for _ in range(NT)]
            for woff, dst in ((W_QC, qc), (W_KC, kc)):
                for ch in range(4):
                    pts = [psum(P, 512) for _ in range(NT)]
                    for kt in range(CT):
                        wt = wpool.tile([P, 512], bf16, tag="wck", name="wck", bufs=3)
                        nc.sync.dma_start(wt[:], wpg[woff + kt * P:woff + (kt + 1) * P, ch * 512:(ch + 1) * 512])
                        for nt in range(NT):
                            nc.tensor.matmul(pts[nt][:], embcT[kt][:, nt * P:(nt + 1) * P],
                                             wt[:], start=(kt == 0), stop=(kt == CT - 1))
                    for nt in range(NT):
                        nc.vector.tensor_copy(dst[nt][:, ch * 512:(ch + 1) * 512], pts[nt][:])

            vct = [big.tile([P, N], bf16, tag="vct", name="vct", bufs=16) for _ in range(CT)]
            for dtg in range(4):
                pts = [psum(P, N) for _ in range(4)]
                for kt in range(CT):
                    wt = wpool.tile([P, 512], bf16, tag="wvk", name="wvk", bufs=3)
                    nc.sync.dma_start(wt[:], wpg[W_VC + kt * P:W_VC + (kt + 1) * P, dtg * 512:(dtg + 1) * 512])
                    for q in range(4):
                        nc.tensor.matmul(pts[q][:], wt[:, q * P:(q + 1) * P], embcT[kt][:],
                                         start=(kt == 0), stop=(kt == CT - 1))
                for q in range(4):
                    nc.vector.tensor_copy(vct[dtg * 4 + q][:], pts[q][:])

            # ---------------- channel attention: A' = attn^T [d, c] -------
            # A' chunks -> DRAM (SBUF can't hold 16MB of A' and E'); global
            # stats accumulate on the fly.
            apd = dram.tile([C, C], bf16, tag="apd", name="apd")
            epd = dram.tile([C, C], bf16, tag="epd", name="epd")
            smsl = sm.tile([P, 64], f32, tag="smsl", name="smsl")
            sqsl = sm.tile([P, 64], f32, tag="sqsl", name="sqsl")
            for dt in range(CT):
                for ch in range(4):
                    pa = psum(P, 512)
                    for nt in range(NT):
                        nc.tensor.matmul(pa[:], kc[nt][:, dt * P:(dt + 1) * P],
                                         qc[nt][:, ch * 512:(ch + 1) * 512],
                                         start=(nt == 0), stop=(nt == NT - 1))
                    idx = dt * 4 + ch
                    sqs = scr.tile([P, 512], bf16, tag="sqs", name="sqs", bufs=2)
                    nc.scalar.activation(sqs[:], pa[:], AF.Square,
                                         accum_out=sqsl[:, idx:idx + 1])
                    apw = scr.tile([P, 512], bf16, tag="apw", name="apw", bufs=3)
                    with nc.allow_low_precision(reason="bf16 evict, f32 accum"):
                        nc.vector.tensor_scalar(apw[:], pa[:], 0.0, 0.0, op0=ADD, op1=ADD,
                                                accum_out=smsl[:, idx:idx + 1])
                    nc.sync.dma_start(apd[dt * P:(dt + 1) * P, ch * 512:(ch + 1) * 512], apw[:])

            # stats -> scale s = 1/sqrt(var+eps), broadcast to [128,1]
            smv = sm.tile([P, 1], f32, tag="smv", name="smv")
            sqv = sm.tile([P, 1], f32, tag="sqv", name="sqv")
            nc.vector.tensor_reduce(smv[:], smsl[:], AX, ADD)
            nc.vector.tensor_reduce(sqv[:], sqsl[:], AX, ADD)
            stot = sm.tile([1, 1], f32, tag="stot", name="stot")
            qtot = sm.tile([1, 1], f32, tag="qtot", name="qtot")
            part_sum(smv[:], stot[:])
            part_sum(sqv[:], qtot[:])
            m2 = sm.tile([1, 1], f32, tag="m2", name="m2")
            t2 = sm.tile([1, 1], f32, tag="t2", name="t2")
            nc.scalar.activation(m2[:], stot[:], AF.Square, scale=1.0 / M_CH)
            nc.scalar.activation(t2[:], qtot[:], AF.Copy, scale=1.0 / M_CH)
            var1 = sm.tile([1, 1], f32, tag="var1", name="var1")
            nc.vector.tensor_tensor(var1[:], t2[:], m2[:], op=SUB)
            sd1 = sm.tile([1, 1], f32, tag="sd1", name="sd1")
            nc.scalar.activation(sd1[:], var1[:], AF.Sqrt, bias=eps11[:])
            s11 = sm.tile([1, 1], f32, tag="s11", name="s11")
            nc.vector.reciprocal(s11[:], sd1[:])
            sbc = sm.tile([P, 1], f32, tag="sbc", name="sbc")
            bcast_col(s11[:], sbc[:])

            # pass A: stream A' from DRAM, exp, accumulate column sums over
            # d (partitions, via ones-lhsT matmul); write E' back to DRAM
            pcs = [psum(1, 512) for _ in range(4)]
            for dt in range(CT):
                apr = scr.tile([P, C], bf16, tag="apr", name="apr", bufs=3)
                nc.sync.dma_start(apr[:], apd[dt * P:(dt + 1) * P, :])
                nc.scalar.activation(apr[:], apr[:], AF.Exp, scale=sbc[:])
                for ch in range(4):
                    nc.tensor.matmul(pcs[ch][:], ones_col[:],
                                     apr[:, ch * 512:(ch + 1) * 512],
                                     start=(dt == 0), stop=(dt == CT - 1))
                nc.sync.dma_start(epd[dt * P:(dt + 1) * P, :], apr[:])
            rr = sm.tile([1, C], f32, tag="rr", name="rr")
            for ch in range(4):
                nc.vector.reciprocal(rr[:, ch * 512:(ch + 1) * 512], pcs[ch][:])
            # transpose [1, C] -> [128, 16] via DRAM bounce
            rb_d = dram.tile([1, C], f32, tag="rb", name="rb")
            nc.sync.dma_start(rb_d[:], rr[:])
            rT = sm.tile([P, CT], f32, tag="rT", name="rT")
            nc.sync.dma_start(rT[:], rb_d[:].rearrange("a (t p) -> (a p) t", p=P))

            # pass B: ctx[c,n] = (E'^T @ VCT) * recip_colsum[c], two groups of
            # 8 PSUM accumulators; E' streamed per d-tile
            ctx_sb = [big.tile([P, N], bf16, tag="ctx", name="ctx", bufs=16) for _ in range(CT)]
            for g in range(2):
                pcxs = [psum(P, N) for _ in range(8)]
                for dt in range(CT):
                    epr = scr.tile([P, C], bf16, tag="apr", name="epr", bufs=3)
                    nc.sync.dma_start(epr[:], epd[dt * P:(dt + 1) * P, :])
                    for k in range(8):
                        ct = g * 8 + k
                        nc.tensor.matmul(pcxs[k][:], epr[:, ct * P:(ct + 1) * P], vct[dt][:],
                                         start=(dt == 0), stop=(dt == CT - 1))
                for k in range(8):
                    ct = g * 8 + k
                    nc.vector.tensor_scalar_mul(ctx_sb[ct][:], pcxs[k][:], rT[:, ct:ct + 1])

            # ---------------- shared K/V over the 4N gathered tokens ------
            wk_sb = [sm.tile([P, E], bf16, tag="wk", name="wk", bufs=4) for _ in range(ET)]
            wv_sb = [sm.tile([P, E], bf16, tag="wv", name="wv", bufs=4) for _ in range(ET)]
            for et in range(ET):
                nc.sync.dma_start(wk_sb[et][:], half(wpg[W_K + et * 32:W_K + (et + 1) * 32, :], b=4))
                nc.sync.dma_start(wv_sb[et][:], half(wpg[W_V + et * 32:W_V + (et + 1) * 32, :], b=4))

            kt_sb = [big.tile([P, 4 * N], bf16, tag="kt", name="kt", bufs=4) for _ in range(ET)]
            for pt in range(ET):
                for j in range(4):
                    pk = psum(P, 512)
                    for et in range(ET):
                        nc.tensor.matmul(pk[:], wk_sb[et][:, pt * P:(pt + 1) * P],
                                         ctx_sb[4 * j + et][:],
                                         start=(et == 0), stop=(et == ET - 1))
                    nc.vector.tensor_copy(kt_sb[pt][:, j * 512:(j + 1) * 512], pk[:])

            vaug = [big.tile([P, H * (D + 1)], bf16, tag="embva", name="vaug", bufs=16, padded_shape=[P, 528]) for _ in range(MT)]
            for mt in range(MT):
                j, q = mt // 4, mt % 4
                pv = psum(P, 512)
                for et in range(ET):
                    nc.tensor.matmul(pv[:], ctx_sb[4 * j + et][:, q * P:(q + 1) * P],
                                     wv_sb[et][:], start=(et == 0), stop=(et == ET - 1))
                va = vaug[mt][:].rearrange("p (h x) -> p h x", x=D + 1)
                nc.vector.tensor_copy(va[:, :, 0:D], pv[:].rearrange("p (h x) -> p h x", x=D))
                nc.any.memset(va[:, :, D:D + 1], 1.0)

            # ---------------- two spatial branches -------------------------
            for br in range(2):
                ebT = [sm.tile([P, N], bf16, tag="ebT", name="ebT", bufs=4) for _ in range(ET)]
                wq_sb = [sm.tile([P, E], bf16, tag="wq", name="wq", bufs=4) for _ in range(ET)]
                wo_sb = [sm.tile([P, E], bf16, tag="wo", name="wo", bufs=4) for _ in range(ET)]
                r_e = R_E0 if br == 0 else R_E1
                for et in range(ET):
                    nc.sync.dma_start(ebT[et][:], half(blob_d[r_e + et * 64:r_e + (et + 1) * 64, :]))
                    nc.sync.dma_start(wq_sb[et][:], wbrg[br * 512 + et * P:br * 512 + (et + 1) * P, :])
                    nc.sync.dma_start(wo_sb[et][:], wbrg[1024 + br * 512 + et * P:1024 + br * 512 + (et + 1) * P, :])

                qt_sb = [sm.tile([P, N], bf16, tag="qt", name="qt", bufs=4) for _ in range(ET)]
                for pt in range(ET):
                    pq = psum(P, N)
                    for et in range(ET):
                        nc.tensor.matmul(pq[:], wq_sb[et][:, pt * P:(pt + 1) * P],
                                         ebT[et][:], start=(et == 0), stop=(et == ET - 1))
                    nc.vector.tensor_copy(qt_sb[pt][:], pq[:])

                ctxT = [sm.tile([P, N], bf16, tag="ctxT", name="ctxT", bufs=8) for _ in range(ET)]
                for h in range(H):
                    pt, off = h // 2, (h % 2) * D
                    lh = big.tile([P, MT * N], bf16, tag="lh", name="lh", bufs=2)
                    hsm = sm.tile([P, MT], f32, tag="hsm", name="hsm", bufs=2)
                    hsq = sm.tile([P, MT], f32, tag="hsq", name="hsq", bufs=2)
                    for mt in range(MT):
                        pl = psum(P, N)
                        nc.tensor.matmul(pl[:], kt_sb[pt][off:off + D, mt * P:(mt + 1) * P],
                                         qt_sb[pt][off:off + D, :], start=True, stop=True)
                        sqs = scr.tile([P, 512], bf16, tag="sqs", name="sqs", bufs=2)
                        nc.scalar.activation(sqs[:], pl[:], AF.Square,
                                             accum_out=hsq[:, mt:mt + 1])
                        with nc.allow_low_precision(reason="bf16 evict, f32 accum"):
                            nc.vector.tensor_scalar(lh[:, mt * N:(mt + 1) * N], pl[:],
                                                    0.0, 0.0, op0=ADD, op1=ADD,
                                                    accum_out=hsm[:, mt:mt + 1])
                    hsmv = sm.tile([P, 1], f32, tag="hsmv", name="hsmv", bufs=2)
                    hsqv = sm.tile([P, 1], f32, tag="hsqv", name="hsqv", bufs=2)
                    nc.vector.tensor_reduce(hsmv[:], hsm[:], AX, ADD)
                    nc.vector.tensor_reduce(hsqv[:], hsq[:], AX, ADD)
                    hst = sm.tile([1, 1], f32, tag="hst", name="hst", bufs=2)
                    hqt = sm.tile([1, 1], f32, tag="hqt", name="hqt", bufs=2)
                    part_sum(hsmv[:], hst[:])
                    part_sum(hsqv[:], hqt[:])
                    hm2 = sm.tile([1, 1], f32, tag="hm2", name="hm2", bufs=2)
                    ht2 = sm.tile([1, 1], f32, tag="ht2", name="ht2", bufs=2)
                    nc.scalar.activation(hm2[:], hst[:], AF.Square, scale=1.0 / M_SP)
                    nc.scalar.activation(ht2[:], hqt[:], AF.Copy, scale=1.0 / M_SP)
                    hvar = sm.tile([1, 1], f32, tag="hvar", name="hvar", bufs=2)
                    nc.vector.tensor_tensor(hvar[:], ht2[:], hm2[:], op=SUB)
                    hsd1 = sm.tile([1, 1], f32, tag="hsd1", name="hsd1", bufs=2)
                    nc.scalar.activation(hsd1[:], hvar[:], AF.Sqrt, bias=eps11[:])
                    hs11 = sm.tile([1, 1], f32, tag="hs11", name="hs11", bufs=2)
                    nc.vector.reciprocal(hs11[:], hsd1[:])
                    hsbc = sm.tile([P, 1], f32, tag="hsbc", name="hsbc", bufs=2)
                    bcast_col(hs11[:], hsbc[:])

                    nc.scalar.activation(lh[:], lh[:], AF.Exp, scale=hsbc[:])
                    es = lh

                    pcx2 = ps.tile([D + 1, N], f32, tag="ps", name="ps")
                    for mt in range(MT):
                        nc.tensor.matmul(pcx2[:], vaug[mt][:, h * (D + 1):(h + 1) * (D + 1)],
                                         es[:, mt * N:(mt + 1) * N],
                                         start=(mt == 0), stop=(mt == MT - 1))
                    rcs = sm.tile([1, N], bf16, tag="rcs", name="rcs", bufs=2)
                    with nc.allow_low_precision(reason="bf16 reciprocal for bcast matmul"):
                        nc.vector.reciprocal(rcs[:], pcx2[D:D + 1, :])
                    prb = psum(D, N)
                    nc.tensor.matmul(prb[:], ones_row64[:], rcs[:], start=True, stop=True)
                    rcb = sm.tile([D, N], f32, tag="rcb", name="rcb", bufs=2)
                    nc.scalar.copy(rcb[:], prb[:])
                    nc.vector.tensor_tensor(ctxT[pt][off:off + D, :], pcx2[0:D, :],
                                            rcb[:], op=MULT)

                for nt2 in range(NT):
                    po = psum(P, E)
                    for pt in range(ET):
                        nc.tensor.matmul(po[:], ctxT[pt][:, nt2 * P:(nt2 + 1) * P],
                                         wo_sb[pt][:], start=(pt == 0), stop=(pt == ET - 1))
                    osb = scr.tile([P, E], bf16, tag="osb", name="osb", bufs=2)
                    nc.vector.tensor_copy(osb[:], po[:])
                    nc.sync.dma_start(out_d[br, nt2 * P:(nt2 + 1) * P, :], osb[:])

    nc.compile()
    return nc


def _get_nc():
    if "nc" not in _cache:
        _cache["nc"] = _build()
    return _cache["nc"]


def kernel(emb1, emb2, emb3, emb4, emb_C,
           Wq1, Wq2, Wq3, Wq4, Wk, Wv, WqC, WkC, WvC,
           Wo1, Wo2, Wo3, Wo4):
    from concourse.bass_utils import run_bass_kernel_spmd
    import os, time
    _tm = bool(os.environ.get("BASSK_TIMING"))
    _t0 = time.perf_counter()

    bf = ml_dtypes.bfloat16
    embs = [np.asarray(e, np.float32) for e in (emb1, emb2, emb3, emb4)]
    Wqs = [np.asarray(w, np.float32) for w in (Wq1, Wq2, Wq3, Wq4)]
    Wos = [np.asarray(w, np.float32) for w in (Wo1, Wo2, Wo3, Wo4)]
    emb_C = np.asarray(emb_C, np.float32)
    WqC, WkC, WvC = (np.asarray(w, np.float32) for w in (WqC, WkC, WvC))
    Wk, Wv = np.asarray(Wk, np.float32), np.asarray(Wv, np.float32)

    blob = np.empty((8, BLOB_ROWS, 1024), bf)

    # shared weight pack -> 8-way shards
    # NB: two-step (.T.astype then bf16 memcpy) — a direct strided-cast
    # assignment into bf16 goes through numpy's slow buffered-iterator path
    wpack = np.empty((WPACK_ROWS, 2048), bf)
    wpack[W_QC:W_QC + C] = WqC.T.astype(bf)
    wpack[W_KC:W_KC + C] = WkC.T.astype(bf)
    wpack[W_VC:W_VC + C] = WvC.T.astype(bf)
    wpack[W_K:W_K + 128].reshape(E, E)[:] = Wk.T.astype(bf)
    wpack[W_V:W_V + 128].reshape(E, E)[:] = Wv.T.astype(bf)
    blob[:, R_WSH:R_SEL, :] = wpack.reshape(8, 1600, 1024)

    # embC carried x1: core c ships half (c%2) of embcT[c//2]; the gathered
    # [4C, N] buffer then holds batch b' contiguously at rows [b'*C,(b'+1)*C)
    for b in range(B):
        ecT = emb_C[b].T.astype(bf)  # [2048, 512]
        blob[2 * b, R_EMBH:R_E0, :].reshape(1024, N)[:] = ecT[0:1024]
        blob[2 * b + 1, R_EMBH:R_E0, :].reshape(1024, N)[:] = ecT[1024:2048]

    sel = np.zeros((8, 4), bf)
    for b in range(B):
        for h in range(2):
            c = 4 * h + b
            sel[c, b] = 1.0
            blob[c, R_E0:R_E1, :].reshape(E, N)[:] = embs[2 * h][b].T.astype(bf)
            blob[c, R_E1:R_WBR, :].reshape(E, N)[:] = embs[2 * h + 1][b].T.astype(bf)
            # quad-gather contribution: rank b of [[0..3],[4..7]] ships
            # [Wq_2h, Wq_2h+1, Wo_2h, Wo_2h+1][b]
            wsrc = (Wqs[2 * h], Wqs[2 * h + 1], Wos[2 * h], Wos[2 * h + 1])[b]
            blob[c, R_WBR:R_WSH, :].reshape(E, E)[:] = wsrc.T.astype(bf)
    blob[:, R_SEL, 0:4] = sel

    in_maps = [{"blob": blob[c]} for c in range(8)]
    if _tm:
        print(f"[timing] host prep: {time.perf_counter()-_t0:.3f}s", file=sys.stderr)
        _t0 = time.perf_counter()

    nc = _get_nc()
    trace = bool(os.environ.get("BASSK_TRACE"))
    try:
        res = run_bass_kernel_spmd(nc, in_maps, core_ids=list(range(8)), trace=trace)
    except ModuleNotFoundError:
        # NTFF profile hook unavailable in this container; run untraced
        res = run_bass_kernel_spmd(nc, in_maps, core_ids=list(range(8)))
    _cache["last_result"] = res
    if _tm:
        print(f"[timing] run_bass_kernel_spmd: {time.perf_counter()-_t0:.3f}s", file=sys.stderr)
        _t0 = time.perf_counter()

    outs = []
    for br in range(4):
        h, j = br // 2, br % 2
        outs.append(np.stack([res.results[4 * h + b]["out"][j].astype(np.float32)
                              for b in range(B)]))
    if _tm:
        print(f"[timing] gather outputs: {time.perf_counter()-_t0:.3f}s", file=sys.stderr)
    return tuple(outs)


if __name__ == "__main__":
    sys.path.insert(0, "/root/problem")
    import reference
    inputs = reference.setup_inputs()
    inputs = {k: np.asarray(v) for k, v in inputs.items()}
    exp = reference.reference(**inputs)
    act = kernel(**inputs)
    for i, (a, e) in enumerate(zip(act, exp)):
        e = np.asarray(e)
        err = np.linalg.norm(a - e) / max(np.linalg.norm(e), 1e-30)
        print(f"out{i + 1}: rel_err={err:.3e}")
